# revision 21
# baseline (speedup 1.0000x reference)
"""DeepSeekV3 latent attention (MLA) Trainium2 Bass kernel.

Sharding: 8 cores = 2 batches x 4 head-groups (4 heads each).
Each core computes, for its (batch b, head-group hg):
  - c_kv = RMSNorm(x_b @ W_DKV.T) * w        (replicated across the 4 hg cores)
  - k_rope / q projections for its 4 heads (weights sliced on the head axis)
  - causal latent attention (no-max-sub softmax, exp/sum form)
  - out_partial = ctx_hg @ out_w[:, hg_cols].T   (row-parallel partial)
Host sums the 4 partials per batch and adds the bias.

Device layout notes: everything is kept "feature-on-partition, token-on-free"
(transposed) so attention scores come out as S^T [tk, tq] and the probs feed
the PV matmul directly with no on-chip transposes of the big tensors.  The
softmax denominator is computed with a ones-column matmul; RoPE's rotate-half
is a signed 64x64 permutation matmul plus two multiplies.
"""

import numpy as np
import ml_dtypes

import concourse.bass as bass
import concourse.tile as tile
from concourse import bacc
from concourse import mybir
from concourse.bass import ts
from concourse.bass_utils import run_bass_kernel_spmd

BF16 = mybir.dt.bfloat16
F32 = mybir.dt.float32
NPBF16 = ml_dtypes.bfloat16

H, HD, RD, LAT = 16, 128, 64, 512
D_IN = 2048
D_OUT = H * HD
HPC = 4  # heads per core
LC = LAT // 128
EPS = 1e-6
THETA = 10000.0
SCALE = 1.0 / float(np.sqrt(np.float32(HD + RD)))
AF = mybir.ActivationFunctionType


def build_mla_nc(T=2048):
    nc = bacc.Bacc("TRN2", target_bir_lowering=False)
    DC = D_IN // 128      # 16 contraction chunks for the x projections
    NT = T // 128         # 128-token tiles
    NJ = T // 512         # 512-token query supertiles
    LC = LAT // 128       # 4 latent chunks

    # ---------------- I/O (all layouts are host-prepared, partition-major) ---
    xT = nc.dram_tensor("xT", [128, DC, T], BF16, kind="ExternalInput")
    xq = nc.dram_tensor("xq", [128, DC, 512], BF16, kind="ExternalInput")
    wdkvT = nc.dram_tensor("wdkvT", [128, DC, LAT], BF16, kind="ExternalInput")
    wkrT = nc.dram_tensor("wkrT", [128, DC, HPC * RD], BF16, kind="ExternalInput")
    wqcT = nc.dram_tensor("wqcT", [128, DC, HPC * HD], BF16, kind="ExternalInput")
    wqrT = nc.dram_tensor("wqrT", [128, DC, HPC * RD], BF16, kind="ExternalInput")
    wuk = nc.dram_tensor("wuk", [128, HPC, LAT], BF16, kind="ExternalInput")
    wuvT = nc.dram_tensor("wuvT", [128, HPC, LC, HD], BF16, kind="ExternalInput")
    owT = nc.dram_tensor("owT", [128, HPC, D_OUT], BF16, kind="ExternalInput")
    kvw = nc.dram_tensor("kvw", [128, LAT], F32, kind="ExternalInput")
    cosT = nc.dram_tensor("cosT", [128, T], BF16, kind="ExternalInput")
    sinT = nc.dram_tensor("sinT", [128, T], BF16, kind="ExternalInput")
    perm = nc.dram_tensor("perm", [128, 128], BF16, kind="ExternalInput")
    masks = nc.dram_tensor("masks", [128, 4, 512], BF16, kind="ExternalInput")
    ones16 = nc.dram_tensor("ones16", [128, 128], BF16, kind="ExternalInput")
    out_p = nc.dram_tensor("out_p", [T, D_OUT], BF16, kind="ExternalOutput")

    with tile.TileContext(nc) as tc:
        with (
            tc.tile_pool(name="persist", bufs=1) as persist,
            tc.tile_pool(name="ps_a", bufs=2, space="PSUM") as ps_a,
            tc.tile_pool(name="ps_b", bufs=2, space="PSUM") as ps_b,
            tc.tile_pool(name="ps_pv", bufs=1, space="PSUM") as ps_pv,
        ):
            ckv_nat = persist.tile([128, NT, LAT], BF16)   # [t%128, ttile, lat]
            ckvT = persist.tile([128, LC, T], BF16)        # [lat%128, lc, t]
            kTrot = persist.tile([128, 2, T], BF16)        # [pairrow, h//2, t]
            qTrot = persist.tile([128, 2, T], BF16)
            qcT = persist.tile([128, HPC, T], BF16)        # [hd, h, t]

            # ============== Phase A: projections + RMSNorm + RoPE ===========
            with (
                tc.tile_pool(name="projw", bufs=1) as projw,
                tc.tile_pool(name="xs", bufs=2) as xs,
                tc.tile_pool(name="wka", bufs=3) as wka,
                tc.tile_pool(name="ckvd", bufs=1, space="DRAM") as ckvd,
            ):
                wdkvT_s = projw.tile([128, DC, LAT], BF16)
                wkrT_s = projw.tile([128, DC, HPC * RD], BF16)
                wqcT_s = projw.tile([128, DC, HPC * HD], BF16)
                wqrT_s = projw.tile([128, DC, HPC * RD], BF16)
                for dc in range(DC):
                    nc.sync.dma_start(wdkvT_s[:, dc, :], wdkvT[:, dc, :])
                kvw_s = projw.tile([128, LAT], F32)
                nc.sync.dma_start(kvw_s, kvw[:, :])
                cos_s = projw.tile([128, T], BF16)
                nc.sync.dma_start(cos_s, cosT[:, :])
                sin_s = projw.tile([128, T], BF16)
                nc.sync.dma_start(sin_s, sinT[:, :])
                perm_s = projw.tile([128, 128], BF16)
                nc.sync.dma_start(perm_s, perm[:, :])
                for dc in range(DC):
                    nc.sync.dma_start(wkrT_s[:, dc, :], wkrT[:, dc, :])
                    nc.sync.dma_start(wqcT_s[:, dc, :], wqcT[:, dc, :])
                    nc.sync.dma_start(wqrT_s[:, dc, :], wqrT[:, dc, :])
                eps_s = projw.tile([128, 1], F32)
                nc.vector.memset(eps_s, EPS)

                # ---- c_kv: this core computes only its 512-token quarter,
                # then an AllGather over the 4 cores sharing this batch
                # rebuilds the full [T, LAT] latent (k_rope/q projections
                # below overlap the collective).
                xqs = xs.tile([128, DC, 512], BF16)
                for dcx in range(DC):
                    nc.scalar.dma_start(xqs[:, dcx, :], xq[:, dcx, :])
                ckv_own = wka.tile([128, 4, LAT], BF16, tag="ckv_own", bufs=1)
                for tt4 in range(4):
                    ps = ps_a.tile([128, 512], F32, tag="mm")
                    for dc in range(DC):
                        nc.tensor.matmul(
                            ps,
                            lhsT=xqs[:, dc, ts(tt4, 128)],
                            rhs=wdkvT_s[:, dc, :],
                            start=(dc == 0),
                            stop=(dc == DC - 1),
                        )
                    sq = wka.tile([128, LAT], BF16, tag="sq", bufs=1)
                    ssum = wka.tile([128, 1], F32, tag="ssum", bufs=2)
                    nc.scalar.activation(sq, ps, AF.Square, accum_out=ssum)
                    rstd = wka.tile([128, 1], F32, tag="rstd", bufs=2)
                    nc.scalar.activation(
                        rstd, ssum, AF.Sqrt, bias=eps_s, scale=1.0 / LAT
                    )
                    nc.vector.reciprocal(rstd, rstd)
                    tmp = wka.tile([128, LAT], BF16, tag="ckvtmp", bufs=2)
                    nc.scalar.activation(tmp, ps, AF.Copy, scale=rstd)
                    nc.vector.tensor_mul(ckv_own[:, tt4, :], tmp, kvw_s)
                ckv_in = ckvd.tile([128, 4, LAT], BF16)
                nc.gpsimd.dma_start(ckv_in, ckv_own)
                ckv_gat = ckvd.tile([4, 128, 4, LAT], BF16)
                nc.gpsimd.collective_compute(
                    "AllGather",
                    mybir.AluOpType.bypass,
                    replica_groups=[[0, 1, 2, 3], [4, 5, 6, 7]],
                    ins=[ckv_in.opt()],
                    outs=[ckv_gat.opt()],
                )
                for g in range(4):
                    nc.gpsimd.dma_start(ckv_nat[:, 4 * g : 4 * g + 4, :], ckv_gat[g])

                def rope_pair(raw_src_psum, dst, rc, jt, tag):
                    # raw_src_psum: [128,512] psum with 2 heads' raw rope rows.
                    raw = wka.tile([128, 512], BF16, tag=f"{tag}_raw", bufs=2)
                    nc.scalar.copy(raw, raw_src_psum)
                    psr = ps_b.tile([128, 512], F32, tag="ps_small")
                    nc.tensor.matmul(psr, lhsT=perm_s, rhs=raw, start=True, stop=True)
                    tmp = wka.tile([128, 512], BF16, tag=f"{tag}_cos", bufs=2)
                    nc.vector.tensor_mul(tmp, raw, cos_s[:, ts(jt, 512)])
                    tmp2 = wka.tile([128, 512], BF16, tag=f"{tag}_sin", bufs=2)
                    nc.vector.tensor_mul(tmp2, psr, sin_s[:, ts(jt, 512)])
                    nc.vector.tensor_add(dst[:, rc, ts(jt, 512)], tmp, tmp2)

                for jt in range(NJ):
                    xts = xs.tile([128, DC, 512], BF16)
                    for q4 in range(4):
                        nc.scalar.dma_start(
                            xts[:, 4 * q4 : 4 * q4 + 4, :],
                            xT[:, 4 * q4 : 4 * q4 + 4, ts(jt, 512)],
                        )

                    # --- k_rope (transposed) + RoPE ---
                    for rc in range(2):
                        ps = ps_a.tile([128, 512], F32, tag="mm")
                        for dc in range(DC):
                            nc.tensor.matmul(
                                ps,
                                lhsT=wkrT_s[:, dc, ts(rc, 128)],
                                rhs=xts[:, dc, :],
                                start=(dc == 0),
                                stop=(dc == DC - 1),
                            )
                        rope_pair(ps, kTrot, rc, jt, "k")

                    # --- q content (transposed) ---
                    for fc in range(HPC):
                        ps = ps_a.tile([128, 512], F32, tag="mm")
                        for dc in range(DC):
                            nc.tensor.matmul(
                                ps,
                                lhsT=wqcT_s[:, dc, ts(fc, 128)],
                                rhs=xts[:, dc, :],
                                start=(dc == 0),
                                stop=(dc == DC - 1),
                            )
                        nc.vector.tensor_copy(qcT[:, fc, ts(jt, 512)], ps)

                    # --- q rope (transposed) + RoPE ---
                    for rc in range(2):
                        ps = ps_a.tile([128, 512], F32, tag="mm")
                        for dc in range(DC):
                            nc.tensor.matmul(
                                ps,
                                lhsT=wqrT_s[:, dc, ts(rc, 128)],
                                rhs=xts[:, dc, :],
                                start=(dc == 0),
                                stop=(dc == DC - 1),
                            )
                        rope_pair(ps, qTrot, rc, jt, "q")

                # transposed c_kv for the QK side (scalar DMA queue: it is
                # idle after the x loads, and phase B weights use sync)
                for tt in range(NT):
                    for lc2 in range(LC):
                        nc.scalar.dma_start(
                            ckvT[:, lc2, ts(tt, 128)],
                            ckv_nat[:, tt, ts(lc2, 128)],
                            transpose=True,
                        )

            # ============== Phase B: attention =============================
            with (
                tc.tile_pool(name="attw", bufs=1) as attw,
                tc.tile_pool(name="qa", bufs=1) as qa_pool,
                tc.tile_pool(name="ctxp", bufs=1) as ctxp,
                tc.tile_pool(name="exps", bufs=10) as exps,
                tc.tile_pool(name="wkb", bufs=3) as wkb,
                tc.tile_pool(name="dnd", bufs=2, space="DRAM") as dnd,
            ):
                ctxT = ctxp.tile([128, HPC, T], BF16)      # [hd, h, t]
                wuk_s = attw.tile([128, HPC, LAT], BF16)
                nc.sync.dma_start(wuk_s, wuk[:, :, :])
                wuvT_s = attw.tile([128, HPC, LC, HD], BF16)
                nc.sync.dma_start(wuvT_s, wuvT[:, :, :, :])
                owT_s = attw.tile([128, HPC, D_OUT], BF16)
                for hc4 in range(HPC):
                    nc.sync.dma_start(owT_s[:, hc4, :], owT[:, hc4, :])
                masks_s = attw.tile([128, 4, 512], BF16)
                nc.sync.dma_start(masks_s, masks[:, :, :])
                ones16_s = attw.tile([128, 128], BF16)
                nc.sync.dma_start(ones16_s, ones16[:, :])

                for h in range(HPC):
                    # absorbed q: qa = W_UK_h.T-contract with q_content
                    qaT = qa_pool.tile([128, LC, T], BF16, tag="qaT")
                    for lc in range(LC):
                        for jt in range(NJ):
                            ps = ps_b.tile([128, 512], F32, tag="ps_small")
                            nc.tensor.matmul(
                                ps,
                                lhsT=wuk_s[:, h, ts(lc, 128)],
                                rhs=qcT[:, h, ts(jt, 512)],
                                start=True,
                                stop=True,
                            )
                            if (lc * NJ + jt) % 2 == 0:
                                nc.vector.tensor_copy(qaT[:, lc, ts(jt, 512)], ps)
                            else:
                                nc.scalar.copy(qaT[:, lc, ts(jt, 512)], ps)

                    hb = (h % 2) * 64
                    rc = h // 2
                    for j in range(NJ):
                        ntk = 4 * (j + 1)
                        ps_ctx = ps_pv.tile([128, LC, 512], F32, tag="ps_ctx")
                        ps_dn = ps_b.tile([1, 512], F32, tag="ps_small")
                        def qk_block(tk):
                            # diagonal blocks: queries left of the block's
                            # first key are fully masked -- skip those columns
                            r = tk - 4 * j
                            q0 = 128 * r if r > 0 else 0
                            ps_s = ps_a.tile([128, 512], F32, tag="mm")
                            for lc in range(LC):
                                nc.tensor.matmul(
                                    ps_s[:, q0:],
                                    lhsT=ckvT[:, lc, ts(tk, 128)],
                                    rhs=qaT[:, lc, 512 * j + q0 : 512 * (j + 1)],
                                    start=(lc == 0),
                                    stop=False,
                                )
                            nc.tensor.matmul(
                                ps_s[:, q0:],
                                lhsT=kTrot[hb : hb + 64, rc, ts(tk, 128)],
                                rhs=qTrot[hb : hb + 64, rc, 512 * j + q0 : 512 * (j + 1)],
                                start=False,
                                stop=True,
                            )
                            ex = exps.tile([128, 512], BF16, tag="exp")
                            nc.scalar.activation(ex[:, q0:], ps_s[:, q0:], AF.Exp, scale=SCALE)
                            if r >= 0:
                                nc.gpsimd.tensor_mul(
                                    ex[:, q0:], ex[:, q0:], masks_s[:, r, q0:]
                                )
                            return ex, q0

                        def pv_block(tk, ex, q0):
                            for lc in range(LC):
                                nc.tensor.matmul(
                                    ps_ctx[:, lc, q0:],
                                    lhsT=ckv_nat[:, tk, ts(lc, 128)],
                                    rhs=ex[:, q0:],
                                    start=(tk == 0),
                                    stop=(tk == ntk - 1),
                                )
                            nc.tensor.matmul(
                                ps_dn[:, q0:],
                                lhsT=ones16_s[:, 0:1],
                                rhs=ex[:, q0:],
                                start=(tk == 0),
                                stop=(tk == ntk - 1),
                            )

                        # software pipeline: PV of pair p runs under QK of
                        # pair p+1, giving exp/mask a full QK-pair to finish
                        pend = None
                        for tk2 in range(ntk // 2):
                            ex0, q00 = qk_block(2 * tk2)
                            ex1, q01 = qk_block(2 * tk2 + 1)
                            if pend is not None:
                                pv_block(*pend[0])
                                pv_block(*pend[1])
                            pend = ((2 * tk2, ex0, q00), (2 * tk2 + 1, ex1, q01))
                        pv_block(*pend[0])
                        pv_block(*pend[1])
                        # reciprocal on the 1-partition row (fast-approx DVE
                        # op, ~18 bits; denominators are positive sums of
                        # exps), then broadcast via a DRAM round-trip
                        dn_r = wkb.tile([1, 512], F32, tag="dr")
                        nc.vector.reciprocal_approx_fast(out=dn_r, in_=ps_dn)
                        dr_d = dnd.tile([1, 512], F32, tag="dr_d")
                        nc.sync.dma_start(dr_d, dn_r)
                        db = wkb.tile([128, 512], F32, tag="db")
                        nc.gpsimd.dma_start(db, dr_d.to_broadcast((128, 512)))
                        # UV: ctxT_h = W_UV_h.T-contract with ctx_lat
                        cl = wkb.tile([128, LC, 512], BF16, tag="ctxlat")
                        for lc in range(LC):
                            if lc % 2 == 0:
                                nc.vector.tensor_copy(cl[:, lc, :], ps_ctx[:, lc, :])
                            else:
                                nc.scalar.copy(cl[:, lc, :], ps_ctx[:, lc, :])
                        ps_uv = ps_b.tile([128, 512], F32, tag="ps_small")
                        for lc in range(LC):
                            nc.tensor.matmul(
                                ps_uv,
                                lhsT=wuvT_s[:, h, lc, :],
                                rhs=cl[:, lc, :],
                                start=(lc == 0),
                                stop=(lc == LC - 1),
                            )
                        # drain PSUM fast with a plain copy; the denominator
                        # multiply happens off the critical path once db lands
                        uvr = wkb.tile([128, 512], BF16, tag="uvr")
                        nc.vector.tensor_copy(uvr, ps_uv)
                        nc.gpsimd.tensor_mul(ctxT[:, h, ts(j, 512)], uvr, db)

                # ============== Phase C: output projection =================
                with tc.tile_pool(name="outs", bufs=3) as outs:
                    for tt in range(NT):
                        ot = outs.tile([128, D_OUT], BF16, tag="ot")
                        for oc in range(D_OUT // 512):
                            ps = ps_a.tile([128, 512], F32, tag="mm")
                            for hc in range(HPC):
                                nc.tensor.matmul(
                                    ps,
                                    lhsT=ctxT[:, hc, ts(tt, 128)],
                                    rhs=owT_s[:, hc, ts(oc, 512)],
                                    start=(hc == 0),
                                    stop=(hc == HPC - 1),
                                )
                            if oc % 2 == 0:
                                nc.vector.tensor_copy(ot[:, ts(oc, 512)], ps)
                            else:
                                nc.scalar.copy(ot[:, ts(oc, 512)], ps)
                        nc.sync.dma_start(out_p[ts(tt, 128), :], ot)

    nc.finalize()
    return nc


def _part_major(a2d):
    """[R, C] -> [128, R//128, C] with partition = R % 128."""
    r, c = a2d.shape
    return np.ascontiguousarray(
        a2d.reshape(r // 128, 128, c).transpose(1, 0, 2)
    )


def make_in_maps(x, W_DKV, kv_norm_w, W_KR, W_Q, W_UK, W_UV, out_w, offset, T):
    """Host-side sharding/layout prep. Returns the 8 per-core input dicts."""
    f32 = np.float32
    x = np.asarray(x, f32)
    W_DKV = np.asarray(W_DKV, f32)
    kv_norm_w = np.asarray(kv_norm_w, f32)
    W_KR = np.asarray(W_KR, f32)
    W_Q = np.asarray(W_Q, f32)
    W_UK = np.asarray(W_UK, f32)
    W_UV = np.asarray(W_UV, f32)
    out_w = np.asarray(out_w, f32)
    offset = int(np.asarray(offset))

    def bf(a):
        return np.ascontiguousarray(a).astype(NPBF16)

    # rope tables, mirroring the reference's f32 arithmetic
    inv_freq = (1.0 / (THETA ** (np.arange(0, RD, 2, dtype=f32) / f32(RD)))).astype(f32)
    pos = np.arange(offset, offset + T, dtype=f32)
    ang = (pos[:, None] * inv_freq[None, :]).astype(f32)     # [T, RD/2]
    ang = np.concatenate([ang, ang], axis=-1)                # [T, RD]
    cos_t = np.cos(ang).T                                    # [RD, T]
    sin_t = np.sin(ang).T
    cosT = np.concatenate([cos_t, cos_t], 0)                 # [128, T]
    sinT = np.concatenate([sin_t, sin_t], 0)

    # signed rotate-half permutation (2 heads per 128 partitions), as lhsT
    M = np.zeros((RD, RD), f32)
    for i in range(RD // 2):
        M[i, i + RD // 2] = -1.0
        M[i + RD // 2, i] = 1.0
    perm128 = np.zeros((128, 128), f32)
    perm128[:64, :64] = M
    perm128[64:, 64:] = M
    perm_lhsT = perm128.T

    # diagonal causal masks: block r masked where (128 r + p) > f
    p_idx = np.arange(128)[:, None]
    f_idx = np.arange(512)[None, :]
    masks = np.stack(
        [(128 * r + p_idx <= f_idx).astype(f32) for r in range(4)], axis=1
    )  # [128, 4, 512]

    kvw = np.broadcast_to(kv_norm_w[None, :], (128, LAT)).astype(f32)
    ones16 = np.ones((128, 128), f32)

    wuk_full = W_UK.reshape(H, HD, LAT)
    wuv_full = W_UV.reshape(H, HD, LAT)

    in_maps = []
    for b in range(2):
        xTb = bf(_part_major(x[b].T))  # [128, DC, T]
        for hg in range(4):
            hs = slice(HPC * hg * HD, HPC * (hg + 1) * HD)          # content rows
            rs = slice(D_OUT + HPC * hg * RD, D_OUT + HPC * (hg + 1) * RD)
            heads = slice(HPC * hg, HPC * (hg + 1))
            wuk_c = wuk_full[heads]                                  # [4,128,512]
            wuv_c = wuv_full[heads]
            in_maps.append(
                {
                    "xT": xTb,
                    "xq": np.ascontiguousarray(xTb[:, :, 512 * hg : 512 * (hg + 1)]),
                    "wdkvT": bf(_part_major(W_DKV.T)),
                    "wkrT": bf(_part_major(W_KR[HPC * hg * RD : HPC * (hg + 1) * RD].T)),
                    "wqcT": bf(_part_major(W_Q[hs].T)),
                    "wqrT": bf(_part_major(W_Q[rs].T)),
                    "wuk": bf(wuk_c.transpose(1, 0, 2)),             # [128,4,512]
                    "wuvT": bf(
                        wuv_c.transpose(0, 2, 1)                     # [4,512,128]
                        .reshape(HPC, LC, 128, HD)
                        .transpose(2, 0, 1, 3)                       # [128,4,4,128]
                    ),
                    "owT": bf(
                        out_w[:, hs].T.reshape(HPC, 128, D_OUT).transpose(1, 0, 2)
                    ),
                    "kvw": np.ascontiguousarray(kvw),
                    "cosT": bf(cosT),
                    "sinT": bf(sinT),
                    "perm": bf(perm_lhsT),
                    "masks": bf(masks),
                    "ones16": bf(ones16),
                }
            )
    return in_maps


_NC_CACHE = {}


def get_nc(T=2048):
    if T not in _NC_CACHE:
        _NC_CACHE[T] = build_mla_nc(T)
    return _NC_CACHE[T]


LAST_RESULTS = None


def kernel(x, W_DKV, kv_norm_w, W_KR, W_Q, W_UK, W_UV, out_w, out_b, offset):
    global LAST_RESULTS
    import os

    x = np.asarray(x, np.float32)
    B, T, _ = x.shape
    nc = get_nc(T)
    in_maps = make_in_maps(
        x, W_DKV, kv_norm_w, W_KR, W_Q, W_UK, W_UV, out_w, offset, T
    )
    trace = os.environ.get("MLA_TRACE", "0") == "1"
    res = run_bass_kernel_spmd(
        nc, in_maps, core_ids=list(range(8)), trace=trace
    )
    LAST_RESULTS = res
    out = np.zeros((B, T, D_OUT), np.float32)
    for c, r in enumerate(res.results):
        out[c // 4] += np.asarray(r["out_p"], np.float32)
    out += np.asarray(out_b, np.float32)[None, None, :]
    return out



# revision 23
# speedup vs baseline: 1.0820x; 1.0820x over previous
"""DeepSeekV3 latent attention (MLA) Trainium2 Bass kernel.

Sharding: 8 cores = 2 batches x 4 head-groups (4 heads each).
Each core computes, for its (batch b, head-group hg):
  - c_kv = RMSNorm(x_b @ W_DKV.T) * w        (replicated across the 4 hg cores)
  - k_rope / q projections for its 4 heads (weights sliced on the head axis)
  - causal latent attention (no-max-sub softmax, exp/sum form)
  - out_partial = ctx_hg @ out_w[:, hg_cols].T   (row-parallel partial)
Host sums the 4 partials per batch and adds the bias.

Device layout notes: everything is kept "feature-on-partition, token-on-free"
(transposed) so attention scores come out as S^T [tk, tq] and the probs feed
the PV matmul directly with no on-chip transposes of the big tensors.  The
softmax denominator is computed with a ones-column matmul; RoPE's rotate-half
is a signed 64x64 permutation matmul plus two multiplies.
"""

import numpy as np
import ml_dtypes

import concourse.bass as bass
import concourse.tile as tile
from concourse import bacc
from concourse import mybir
from concourse.bass import ts
from concourse.bass_utils import run_bass_kernel_spmd

BF16 = mybir.dt.bfloat16
F32 = mybir.dt.float32
NPBF16 = ml_dtypes.bfloat16

H, HD, RD, LAT = 16, 128, 64, 512
D_IN = 2048
D_OUT = H * HD
HPC = 4  # heads per core
LC = LAT // 128
EPS = 1e-6
THETA = 10000.0
SCALE = 1.0 / float(np.sqrt(np.float32(HD + RD)))
AF = mybir.ActivationFunctionType


def build_mla_nc(T=2048):
    nc = bacc.Bacc("TRN2", target_bir_lowering=False)
    DC = D_IN // 128      # 16 contraction chunks for the x projections
    NT = T // 128         # 128-token tiles
    NJ = T // 512         # 512-token query supertiles
    LC = LAT // 128       # 4 latent chunks

    # ---------------- I/O (all layouts are host-prepared, partition-major) ---
    xT = nc.dram_tensor("xT", [128, DC, T], BF16, kind="ExternalInput")
    xq = nc.dram_tensor("xq", [128, DC, 512], BF16, kind="ExternalInput")
    wdkvT = nc.dram_tensor("wdkvT", [128, DC, LAT], BF16, kind="ExternalInput")
    wkrT = nc.dram_tensor("wkrT", [128, DC, HPC * RD], BF16, kind="ExternalInput")
    wqcT = nc.dram_tensor("wqcT", [128, DC, HPC * HD], BF16, kind="ExternalInput")
    wqrT = nc.dram_tensor("wqrT", [128, DC, HPC * RD], BF16, kind="ExternalInput")
    wuk = nc.dram_tensor("wuk", [128, HPC, LAT], BF16, kind="ExternalInput")
    wuvT = nc.dram_tensor("wuvT", [128, HPC, LC, HD], BF16, kind="ExternalInput")
    owT = nc.dram_tensor("owT", [128, HPC, D_OUT], BF16, kind="ExternalInput")
    kvw = nc.dram_tensor("kvw", [128, LAT], F32, kind="ExternalInput")
    cosT = nc.dram_tensor("cosT", [128, T], BF16, kind="ExternalInput")
    sinT = nc.dram_tensor("sinT", [128, T], BF16, kind="ExternalInput")
    perm = nc.dram_tensor("perm", [128, 128], BF16, kind="ExternalInput")
    masks = nc.dram_tensor("masks", [128, 4, 512], BF16, kind="ExternalInput")
    ones16 = nc.dram_tensor("ones16", [128, 128], BF16, kind="ExternalInput")
    out_p = nc.dram_tensor("out_p", [T, D_OUT], BF16, kind="ExternalOutput")

    with tile.TileContext(nc) as tc:
        with (
            tc.tile_pool(name="persist", bufs=1) as persist,
            tc.tile_pool(name="ps_a", bufs=2, space="PSUM") as ps_a,
            tc.tile_pool(name="ps_b", bufs=2, space="PSUM") as ps_b,
            tc.tile_pool(name="ps_pv", bufs=1, space="PSUM") as ps_pv,
        ):
            ckv_nat = persist.tile([128, NT, LAT], BF16)   # [t%128, ttile, lat]
            ckvT = persist.tile([128, LC, T], BF16)        # [lat%128, lc, t]
            kTrot = persist.tile([128, 2, T], BF16)        # [pairrow, h//2, t]
            qTrot = persist.tile([128, 2, T], BF16)
            qcT = persist.tile([128, HPC, T], BF16)        # [hd, h, t]

            # ============== Phase A: projections + RMSNorm + RoPE ===========
            with (
                tc.tile_pool(name="projw", bufs=1) as projw,
                tc.tile_pool(name="xs", bufs=2) as xs,
                tc.tile_pool(name="wka", bufs=3) as wka,
                tc.tile_pool(name="ckvd", bufs=1, space="DRAM") as ckvd,
            ):
                wdkvT_s = projw.tile([128, DC, LAT], BF16)
                wkrT_s = projw.tile([128, DC, HPC * RD], BF16)
                wqcT_s = projw.tile([128, DC, HPC * HD], BF16)
                wqrT_s = projw.tile([128, DC, HPC * RD], BF16)
                for dc in range(DC):
                    nc.sync.dma_start(wdkvT_s[:, dc, :], wdkvT[:, dc, :])
                kvw_s = projw.tile([128, LAT], F32)
                nc.sync.dma_start(kvw_s, kvw[:, :])
                cos_s = projw.tile([128, T], BF16)
                nc.sync.dma_start(cos_s, cosT[:, :])
                sin_s = projw.tile([128, T], BF16)
                nc.sync.dma_start(sin_s, sinT[:, :])
                perm_s = projw.tile([128, 128], BF16)
                nc.sync.dma_start(perm_s, perm[:, :])
                for dc in range(DC):
                    nc.sync.dma_start(wkrT_s[:, dc, :], wkrT[:, dc, :])
                    nc.sync.dma_start(wqcT_s[:, dc, :], wqcT[:, dc, :])
                    nc.sync.dma_start(wqrT_s[:, dc, :], wqrT[:, dc, :])
                eps_s = projw.tile([128, 1], F32)
                nc.vector.memset(eps_s, EPS)

                # ---- c_kv: this core computes only its 512-token quarter,
                # then an AllGather over the 4 cores sharing this batch
                # rebuilds the full [T, LAT] latent (k_rope/q projections
                # below overlap the collective).
                xqs = xs.tile([128, DC, 512], BF16)
                for dcx in range(DC):
                    nc.scalar.dma_start(xqs[:, dcx, :], xq[:, dcx, :])
                ckv_own = wka.tile([128, 4, LAT], BF16, tag="ckv_own", bufs=1)
                for tt4 in range(4):
                    ps = ps_a.tile([128, 512], F32, tag="mm")
                    for dc in range(DC):
                        nc.tensor.matmul(
                            ps,
                            lhsT=xqs[:, dc, ts(tt4, 128)],
                            rhs=wdkvT_s[:, dc, :],
                            start=(dc == 0),
                            stop=(dc == DC - 1),
                        )
                    sq = wka.tile([128, LAT], BF16, tag="sq", bufs=1)
                    ssum = wka.tile([128, 1], F32, tag="ssum", bufs=2)
                    nc.scalar.activation(sq, ps, AF.Square, accum_out=ssum)
                    rstd = wka.tile([128, 1], F32, tag="rstd", bufs=2)
                    nc.scalar.activation(
                        rstd, ssum, AF.Sqrt, bias=eps_s, scale=1.0 / LAT
                    )
                    nc.vector.reciprocal(rstd, rstd)
                    tmp = wka.tile([128, LAT], BF16, tag="ckvtmp", bufs=2)
                    nc.scalar.activation(tmp, ps, AF.Copy, scale=rstd)
                    nc.vector.tensor_mul(ckv_own[:, tt4, :], tmp, kvw_s)
                ckv_in = ckvd.tile([128, 4, LAT], BF16)
                nc.gpsimd.dma_start(ckv_in, ckv_own)
                ckv_gat = ckvd.tile([4, 128, 4, LAT], BF16)
                nc.gpsimd.collective_compute(
                    "AllGather",
                    mybir.AluOpType.bypass,
                    replica_groups=[[0, 1, 2, 3], [4, 5, 6, 7]],
                    ins=[ckv_in.opt()],
                    outs=[ckv_gat.opt()],
                )
                for g in range(4):
                    nc.gpsimd.dma_start(ckv_nat[:, 4 * g : 4 * g + 4, :], ckv_gat[g])

                def rope_pair(raw_src_psum, dst, rc, jt, tag):
                    # raw_src_psum: [128,512] psum with 2 heads' raw rope rows.
                    raw = wka.tile([128, 512], BF16, tag=f"{tag}_raw", bufs=2)
                    nc.scalar.copy(raw, raw_src_psum)
                    psr = ps_b.tile([128, 512], F32, tag="ps_small")
                    nc.tensor.matmul(psr, lhsT=perm_s, rhs=raw, start=True, stop=True)
                    tmp = wka.tile([128, 512], BF16, tag=f"{tag}_cos", bufs=2)
                    nc.vector.tensor_mul(tmp, raw, cos_s[:, ts(jt, 512)])
                    tmp2 = wka.tile([128, 512], BF16, tag=f"{tag}_sin", bufs=2)
                    nc.vector.tensor_mul(tmp2, psr, sin_s[:, ts(jt, 512)])
                    nc.vector.tensor_add(dst[:, rc, ts(jt, 512)], tmp, tmp2)

                for jt in range(NJ):
                    xts = xs.tile([128, DC, 512], BF16)
                    for q4 in range(4):
                        nc.scalar.dma_start(
                            xts[:, 4 * q4 : 4 * q4 + 4, :],
                            xT[:, 4 * q4 : 4 * q4 + 4, ts(jt, 512)],
                        )

                    # --- k_rope (transposed) + RoPE ---
                    for rc in range(2):
                        ps = ps_a.tile([128, 512], F32, tag="mm")
                        for dc in range(DC):
                            nc.tensor.matmul(
                                ps,
                                lhsT=wkrT_s[:, dc, ts(rc, 128)],
                                rhs=xts[:, dc, :],
                                start=(dc == 0),
                                stop=(dc == DC - 1),
                            )
                        rope_pair(ps, kTrot, rc, jt, "k")

                    # --- q content (transposed) ---
                    for fc in range(HPC):
                        ps = ps_a.tile([128, 512], F32, tag="mm")
                        for dc in range(DC):
                            nc.tensor.matmul(
                                ps,
                                lhsT=wqcT_s[:, dc, ts(fc, 128)],
                                rhs=xts[:, dc, :],
                                start=(dc == 0),
                                stop=(dc == DC - 1),
                            )
                        nc.vector.tensor_copy(qcT[:, fc, ts(jt, 512)], ps)

                    # --- q rope (transposed) + RoPE ---
                    for rc in range(2):
                        ps = ps_a.tile([128, 512], F32, tag="mm")
                        for dc in range(DC):
                            nc.tensor.matmul(
                                ps,
                                lhsT=wqrT_s[:, dc, ts(rc, 128)],
                                rhs=xts[:, dc, :],
                                start=(dc == 0),
                                stop=(dc == DC - 1),
                            )
                        rope_pair(ps, qTrot, rc, jt, "q")

                # transposed c_kv for the QK side. Must live on the sync
                # queue: these block on the AllGather, and the scalar/vector
                # engine streams carry phase-B compute that cannot queue
                # behind them (engine queues are in-order).
                for tt in range(NT):
                    for lc2 in range(LC):
                        nc.sync.dma_start(
                            ckvT[:, lc2, ts(tt, 128)],
                            ckv_nat[:, tt, ts(lc2, 128)],
                            transpose=True,
                        )

            # ============== Phase B: attention =============================
            with (
                tc.tile_pool(name="attw", bufs=1) as attw,
                tc.tile_pool(name="qa", bufs=1) as qa_pool,
                tc.tile_pool(name="ctxp", bufs=1) as ctxp,
                tc.tile_pool(name="exps", bufs=10) as exps,
                tc.tile_pool(name="wkb", bufs=3) as wkb,
                tc.tile_pool(name="dnd", bufs=2, space="DRAM") as dnd,
            ):
                ctxT = ctxp.tile([128, HPC, T], BF16)      # [hd, h, t]
                wuk_s = attw.tile([128, HPC, LAT], BF16)
                nc.scalar.dma_start(wuk_s, wuk[:, :, :])
                wuvT_s = attw.tile([128, HPC, LC, HD], BF16)
                nc.scalar.dma_start(wuvT_s, wuvT[:, :, :, :])
                owT_s = attw.tile([128, HPC, D_OUT], BF16)
                for hc4 in range(HPC):
                    nc.scalar.dma_start(owT_s[:, hc4, :], owT[:, hc4, :])
                masks_s = attw.tile([128, 4, 512], BF16)
                nc.scalar.dma_start(masks_s, masks[:, :, :])
                ones16_s = attw.tile([128, 128], BF16)
                nc.scalar.dma_start(ones16_s, ones16[:, :])

                for h in range(HPC):
                    # absorbed q: qa = W_UK_h.T-contract with q_content
                    qaT = qa_pool.tile([128, LC, T], BF16, tag="qaT")
                    for lc in range(LC):
                        for jt in range(NJ):
                            ps = ps_b.tile([128, 512], F32, tag="ps_small")
                            nc.tensor.matmul(
                                ps,
                                lhsT=wuk_s[:, h, ts(lc, 128)],
                                rhs=qcT[:, h, ts(jt, 512)],
                                start=True,
                                stop=True,
                            )
                            if (lc * NJ + jt) % 2 == 0:
                                nc.vector.tensor_copy(qaT[:, lc, ts(jt, 512)], ps)
                            else:
                                nc.scalar.copy(qaT[:, lc, ts(jt, 512)], ps)

                    hb = (h % 2) * 64
                    rc = h // 2
                    for j in range(NJ):
                        ntk = 4 * (j + 1)
                        ps_ctx = ps_pv.tile([128, LC, 512], F32, tag="ps_ctx")
                        ps_dn = ps_b.tile([1, 512], F32, tag="ps_small")
                        def qk_block(tk):
                            # diagonal blocks: queries left of the block's
                            # first key are fully masked -- skip those columns
                            r = tk - 4 * j
                            q0 = 128 * r if r > 0 else 0
                            ps_s = ps_a.tile([128, 512], F32, tag="mm")
                            for lc in range(LC):
                                nc.tensor.matmul(
                                    ps_s[:, q0:],
                                    lhsT=ckvT[:, lc, ts(tk, 128)],
                                    rhs=qaT[:, lc, 512 * j + q0 : 512 * (j + 1)],
                                    start=(lc == 0),
                                    stop=False,
                                )
                            nc.tensor.matmul(
                                ps_s[:, q0:],
                                lhsT=kTrot[hb : hb + 64, rc, ts(tk, 128)],
                                rhs=qTrot[hb : hb + 64, rc, 512 * j + q0 : 512 * (j + 1)],
                                start=False,
                                stop=True,
                            )
                            ex = exps.tile([128, 512], BF16, tag="exp")
                            nc.scalar.activation(ex[:, q0:], ps_s[:, q0:], AF.Exp, scale=SCALE)
                            if r >= 0:
                                nc.gpsimd.tensor_mul(
                                    ex[:, q0:], ex[:, q0:], masks_s[:, r, q0:]
                                )
                            return ex, q0

                        def pv_block(tk, ex, q0):
                            for lc in range(LC):
                                nc.tensor.matmul(
                                    ps_ctx[:, lc, q0:],
                                    lhsT=ckv_nat[:, tk, ts(lc, 128)],
                                    rhs=ex[:, q0:],
                                    start=(tk == 0),
                                    stop=(tk == ntk - 1),
                                )
                            nc.tensor.matmul(
                                ps_dn[:, q0:],
                                lhsT=ones16_s[:, 0:1],
                                rhs=ex[:, q0:],
                                start=(tk == 0),
                                stop=(tk == ntk - 1),
                            )

                        # software pipeline: PV of pair p runs under QK of
                        # pair p+1, giving exp/mask a full QK-pair to finish
                        pend = None
                        for tk2 in range(ntk // 2):
                            ex0, q00 = qk_block(2 * tk2)
                            ex1, q01 = qk_block(2 * tk2 + 1)
                            if pend is not None:
                                pv_block(*pend[0])
                                pv_block(*pend[1])
                            pend = ((2 * tk2, ex0, q00), (2 * tk2 + 1, ex1, q01))
                        pv_block(*pend[0])
                        pv_block(*pend[1])
                        # reciprocal on the 1-partition row (fast-approx DVE
                        # op, ~18 bits; denominators are positive sums of
                        # exps), then broadcast via a DRAM round-trip
                        dn_r = wkb.tile([1, 512], F32, tag="dr")
                        nc.vector.reciprocal_approx_fast(out=dn_r, in_=ps_dn)
                        dr_d = dnd.tile([1, 512], F32, tag="dr_d")
                        nc.sync.dma_start(dr_d, dn_r)
                        db = wkb.tile([128, 512], F32, tag="db")
                        nc.gpsimd.dma_start(db, dr_d.to_broadcast((128, 512)))
                        # UV: ctxT_h = W_UV_h.T-contract with ctx_lat
                        cl = wkb.tile([128, LC, 512], BF16, tag="ctxlat")
                        for lc in range(LC):
                            if lc % 2 == 0:
                                nc.vector.tensor_copy(cl[:, lc, :], ps_ctx[:, lc, :])
                            else:
                                nc.scalar.copy(cl[:, lc, :], ps_ctx[:, lc, :])
                        ps_uv = ps_b.tile([128, 512], F32, tag="ps_small")
                        for lc in range(LC):
                            nc.tensor.matmul(
                                ps_uv,
                                lhsT=wuvT_s[:, h, lc, :],
                                rhs=cl[:, lc, :],
                                start=(lc == 0),
                                stop=(lc == LC - 1),
                            )
                        # drain PSUM fast with a plain copy; the denominator
                        # multiply happens off the critical path once db lands
                        uvr = wkb.tile([128, 512], BF16, tag="uvr")
                        nc.vector.tensor_copy(uvr, ps_uv)
                        nc.gpsimd.tensor_mul(ctxT[:, h, ts(j, 512)], uvr, db)

                # ============== Phase C: output projection =================
                with tc.tile_pool(name="outs", bufs=3) as outs:
                    for tt in range(NT):
                        ot = outs.tile([128, D_OUT], BF16, tag="ot")
                        for oc in range(D_OUT // 512):
                            ps = ps_a.tile([128, 512], F32, tag="mm")
                            for hc in range(HPC):
                                nc.tensor.matmul(
                                    ps,
                                    lhsT=ctxT[:, hc, ts(tt, 128)],
                                    rhs=owT_s[:, hc, ts(oc, 512)],
                                    start=(hc == 0),
                                    stop=(hc == HPC - 1),
                                )
                            if oc % 2 == 0:
                                nc.vector.tensor_copy(ot[:, ts(oc, 512)], ps)
                            else:
                                nc.scalar.copy(ot[:, ts(oc, 512)], ps)
                        nc.sync.dma_start(out_p[ts(tt, 128), :], ot)

    nc.finalize()
    return nc


def _part_major(a2d):
    """[R, C] -> [128, R//128, C] with partition = R % 128."""
    r, c = a2d.shape
    return np.ascontiguousarray(
        a2d.reshape(r // 128, 128, c).transpose(1, 0, 2)
    )


def make_in_maps(x, W_DKV, kv_norm_w, W_KR, W_Q, W_UK, W_UV, out_w, offset, T):
    """Host-side sharding/layout prep. Returns the 8 per-core input dicts."""
    f32 = np.float32
    x = np.asarray(x, f32)
    W_DKV = np.asarray(W_DKV, f32)
    kv_norm_w = np.asarray(kv_norm_w, f32)
    W_KR = np.asarray(W_KR, f32)
    W_Q = np.asarray(W_Q, f32)
    W_UK = np.asarray(W_UK, f32)
    W_UV = np.asarray(W_UV, f32)
    out_w = np.asarray(out_w, f32)
    offset = int(np.asarray(offset))

    def bf(a):
        return np.ascontiguousarray(a).astype(NPBF16)

    # rope tables, mirroring the reference's f32 arithmetic
    inv_freq = (1.0 / (THETA ** (np.arange(0, RD, 2, dtype=f32) / f32(RD)))).astype(f32)
    pos = np.arange(offset, offset + T, dtype=f32)
    ang = (pos[:, None] * inv_freq[None, :]).astype(f32)     # [T, RD/2]
    ang = np.concatenate([ang, ang], axis=-1)                # [T, RD]
    cos_t = np.cos(ang).T                                    # [RD, T]
    sin_t = np.sin(ang).T
    cosT = np.concatenate([cos_t, cos_t], 0)                 # [128, T]
    sinT = np.concatenate([sin_t, sin_t], 0)

    # signed rotate-half permutation (2 heads per 128 partitions), as lhsT
    M = np.zeros((RD, RD), f32)
    for i in range(RD // 2):
        M[i, i + RD // 2] = -1.0
        M[i + RD // 2, i] = 1.0
    perm128 = np.zeros((128, 128), f32)
    perm128[:64, :64] = M
    perm128[64:, 64:] = M
    perm_lhsT = perm128.T

    # diagonal causal masks: block r masked where (128 r + p) > f
    p_idx = np.arange(128)[:, None]
    f_idx = np.arange(512)[None, :]
    masks = np.stack(
        [(128 * r + p_idx <= f_idx).astype(f32) for r in range(4)], axis=1
    )  # [128, 4, 512]

    kvw = np.broadcast_to(kv_norm_w[None, :], (128, LAT)).astype(f32)
    ones16 = np.ones((128, 128), f32)

    wuk_full = W_UK.reshape(H, HD, LAT)
    wuv_full = W_UV.reshape(H, HD, LAT)

    in_maps = []
    for b in range(2):
        xTb = bf(_part_major(x[b].T))  # [128, DC, T]
        for hg in range(4):
            hs = slice(HPC * hg * HD, HPC * (hg + 1) * HD)          # content rows
            rs = slice(D_OUT + HPC * hg * RD, D_OUT + HPC * (hg + 1) * RD)
            heads = slice(HPC * hg, HPC * (hg + 1))
            wuk_c = wuk_full[heads]                                  # [4,128,512]
            wuv_c = wuv_full[heads]
            in_maps.append(
                {
                    "xT": xTb,
                    "xq": np.ascontiguousarray(xTb[:, :, 512 * hg : 512 * (hg + 1)]),
                    "wdkvT": bf(_part_major(W_DKV.T)),
                    "wkrT": bf(_part_major(W_KR[HPC * hg * RD : HPC * (hg + 1) * RD].T)),
                    "wqcT": bf(_part_major(W_Q[hs].T)),
                    "wqrT": bf(_part_major(W_Q[rs].T)),
                    "wuk": bf(wuk_c.transpose(1, 0, 2)),             # [128,4,512]
                    "wuvT": bf(
                        wuv_c.transpose(0, 2, 1)                     # [4,512,128]
                        .reshape(HPC, LC, 128, HD)
                        .transpose(2, 0, 1, 3)                       # [128,4,4,128]
                    ),
                    "owT": bf(
                        out_w[:, hs].T.reshape(HPC, 128, D_OUT).transpose(1, 0, 2)
                    ),
                    "kvw": np.ascontiguousarray(kvw),
                    "cosT": bf(cosT),
                    "sinT": bf(sinT),
                    "perm": bf(perm_lhsT),
                    "masks": bf(masks),
                    "ones16": bf(ones16),
                }
            )
    return in_maps


_NC_CACHE = {}


def get_nc(T=2048):
    if T not in _NC_CACHE:
        _NC_CACHE[T] = build_mla_nc(T)
    return _NC_CACHE[T]


LAST_RESULTS = None


def kernel(x, W_DKV, kv_norm_w, W_KR, W_Q, W_UK, W_UV, out_w, out_b, offset):
    global LAST_RESULTS
    import os

    x = np.asarray(x, np.float32)
    B, T, _ = x.shape
    nc = get_nc(T)
    in_maps = make_in_maps(
        x, W_DKV, kv_norm_w, W_KR, W_Q, W_UK, W_UV, out_w, offset, T
    )
    trace = os.environ.get("MLA_TRACE", "0") == "1"
    res = run_bass_kernel_spmd(
        nc, in_maps, core_ids=list(range(8)), trace=trace
    )
    LAST_RESULTS = res
    out = np.zeros((B, T, D_OUT), np.float32)
    for c, r in enumerate(res.results):
        out[c // 4] += np.asarray(r["out_p"], np.float32)
    out += np.asarray(out_b, np.float32)[None, None, :]
    return out



# revision 24
# speedup vs baseline: 1.1148x; 1.0303x over previous
"""DeepSeekV3 latent attention (MLA) Trainium2 Bass kernel.

Sharding: 8 cores = 2 batches x 4 head-groups (4 heads each).
Each core computes, for its (batch b, head-group hg):
  - c_kv = RMSNorm(x_b @ W_DKV.T) * w        (replicated across the 4 hg cores)
  - k_rope / q projections for its 4 heads (weights sliced on the head axis)
  - causal latent attention (no-max-sub softmax, exp/sum form)
  - out_partial = ctx_hg @ out_w[:, hg_cols].T   (row-parallel partial)
Host sums the 4 partials per batch and adds the bias.

Device layout notes: everything is kept "feature-on-partition, token-on-free"
(transposed) so attention scores come out as S^T [tk, tq] and the probs feed
the PV matmul directly with no on-chip transposes of the big tensors.  The
softmax denominator is computed with a ones-column matmul; RoPE's rotate-half
is a signed 64x64 permutation matmul plus two multiplies.
"""

import numpy as np
import ml_dtypes

import concourse.bass as bass
import concourse.tile as tile
from concourse import bacc
from concourse import mybir
from concourse.bass import ts
from concourse.bass_utils import run_bass_kernel_spmd

BF16 = mybir.dt.bfloat16
F32 = mybir.dt.float32
NPBF16 = ml_dtypes.bfloat16

H, HD, RD, LAT = 16, 128, 64, 512
D_IN = 2048
D_OUT = H * HD
HPC = 4  # heads per core
LC = LAT // 128
EPS = 1e-6
THETA = 10000.0
SCALE = 1.0 / float(np.sqrt(np.float32(HD + RD)))
AF = mybir.ActivationFunctionType


def build_mla_nc(T=2048):
    nc = bacc.Bacc("TRN2", target_bir_lowering=False)
    DC = D_IN // 128      # 16 contraction chunks for the x projections
    NT = T // 128         # 128-token tiles
    NJ = T // 512         # 512-token query supertiles
    LC = LAT // 128       # 4 latent chunks

    # ---------------- I/O (all layouts are host-prepared, partition-major) ---
    xT = nc.dram_tensor("xT", [128, DC, T], BF16, kind="ExternalInput")
    xq = nc.dram_tensor("xq", [128, DC, 512], BF16, kind="ExternalInput")
    wdkvT = nc.dram_tensor("wdkvT", [128, DC, LAT], BF16, kind="ExternalInput")
    wkrT = nc.dram_tensor("wkrT", [128, DC, HPC * RD], BF16, kind="ExternalInput")
    wqcT = nc.dram_tensor("wqcT", [128, DC, HPC * HD], BF16, kind="ExternalInput")
    wqrT = nc.dram_tensor("wqrT", [128, DC, HPC * RD], BF16, kind="ExternalInput")
    wuk = nc.dram_tensor("wuk", [128, HPC, LAT], BF16, kind="ExternalInput")
    wuvT = nc.dram_tensor("wuvT", [128, HPC, LC, HD], BF16, kind="ExternalInput")
    owT = nc.dram_tensor("owT", [128, HPC, D_OUT], BF16, kind="ExternalInput")
    kvw = nc.dram_tensor("kvw", [128, LAT], F32, kind="ExternalInput")
    cosT = nc.dram_tensor("cosT", [128, T], BF16, kind="ExternalInput")
    sinT = nc.dram_tensor("sinT", [128, T], BF16, kind="ExternalInput")
    perm = nc.dram_tensor("perm", [128, 128], BF16, kind="ExternalInput")
    masks = nc.dram_tensor("masks", [128, 4, 512], BF16, kind="ExternalInput")
    ones16 = nc.dram_tensor("ones16", [128, 128], BF16, kind="ExternalInput")
    ident = nc.dram_tensor("ident", [128, 128], BF16, kind="ExternalInput")
    out_p = nc.dram_tensor("out_p", [T, D_OUT], BF16, kind="ExternalOutput")

    with tile.TileContext(nc) as tc:
        with (
            tc.tile_pool(name="persist", bufs=1) as persist,
            tc.tile_pool(name="ps_a", bufs=2, space="PSUM") as ps_a,
            tc.tile_pool(name="ps_b", bufs=2, space="PSUM") as ps_b,
            tc.tile_pool(name="ps_pv", bufs=1, space="PSUM") as ps_pv,
        ):
            ckv_nat = persist.tile([128, NT, LAT], BF16)   # [t%128, ttile, lat]
            ckvT = persist.tile([128, LC, T], BF16)        # [lat%128, lc, t]
            kTrot = persist.tile([128, 2, T], BF16)        # [pairrow, h//2, t]
            qTrot = persist.tile([128, 2, T], BF16)
            qcT = persist.tile([128, HPC, T], BF16)        # [hd, h, t]
            ident_s = persist.tile([128, 128], BF16)
            nc.sync.dma_start(ident_s, ident[:, :])

            # ============== Phase A: projections + RMSNorm + RoPE ===========
            with (
                tc.tile_pool(name="projw", bufs=1) as projw,
                tc.tile_pool(name="xs", bufs=2) as xs,
                tc.tile_pool(name="wka", bufs=3) as wka,
                tc.tile_pool(name="ckvd", bufs=1, space="DRAM") as ckvd,
            ):
                wdkvT_s = projw.tile([128, DC, LAT], BF16)
                wkrT_s = projw.tile([128, DC, HPC * RD], BF16)
                wqcT_s = projw.tile([128, DC, HPC * HD], BF16)
                wqrT_s = projw.tile([128, DC, HPC * RD], BF16)
                for dc in range(DC):
                    nc.sync.dma_start(wdkvT_s[:, dc, :], wdkvT[:, dc, :])
                kvw_s = projw.tile([128, LAT], F32)
                nc.sync.dma_start(kvw_s, kvw[:, :])
                cos_s = projw.tile([128, T], BF16)
                nc.sync.dma_start(cos_s, cosT[:, :])
                sin_s = projw.tile([128, T], BF16)
                nc.sync.dma_start(sin_s, sinT[:, :])
                perm_s = projw.tile([128, 128], BF16)
                nc.sync.dma_start(perm_s, perm[:, :])
                for dc in range(DC):
                    nc.sync.dma_start(wkrT_s[:, dc, :], wkrT[:, dc, :])
                    nc.sync.dma_start(wqcT_s[:, dc, :], wqcT[:, dc, :])
                    nc.sync.dma_start(wqrT_s[:, dc, :], wqrT[:, dc, :])
                eps_s = projw.tile([128, 1], F32)
                nc.vector.memset(eps_s, EPS)

                # ---- c_kv: this core computes only its 512-token quarter,
                # then an AllGather over the 4 cores sharing this batch
                # rebuilds the full [T, LAT] latent (k_rope/q projections
                # below overlap the collective).
                xqs = xs.tile([128, DC, 512], BF16)
                for dcx in range(DC):
                    nc.scalar.dma_start(xqs[:, dcx, :], xq[:, dcx, :])
                ckv_own = wka.tile([128, 4, LAT], BF16, tag="ckv_own", bufs=1)
                for tt4 in range(4):
                    ps = ps_a.tile([128, 512], F32, tag="mm")
                    for dc in range(DC):
                        nc.tensor.matmul(
                            ps,
                            lhsT=xqs[:, dc, ts(tt4, 128)],
                            rhs=wdkvT_s[:, dc, :],
                            start=(dc == 0),
                            stop=(dc == DC - 1),
                        )
                    sq = wka.tile([128, LAT], BF16, tag="sq", bufs=1)
                    ssum = wka.tile([128, 1], F32, tag="ssum", bufs=2)
                    nc.scalar.activation(sq, ps, AF.Square, accum_out=ssum)
                    rstd = wka.tile([128, 1], F32, tag="rstd", bufs=2)
                    nc.scalar.activation(
                        rstd, ssum, AF.Sqrt, bias=eps_s, scale=1.0 / LAT
                    )
                    nc.vector.reciprocal(rstd, rstd)
                    tmp = wka.tile([128, LAT], BF16, tag="ckvtmp", bufs=2)
                    nc.scalar.activation(tmp, ps, AF.Copy, scale=rstd)
                    nc.vector.tensor_mul(ckv_own[:, tt4, :], tmp, kvw_s)
                ckv_in = ckvd.tile([128, 4, LAT], BF16)
                nc.gpsimd.dma_start(ckv_in, ckv_own)
                ckv_gat = ckvd.tile([4, 128, 4, LAT], BF16)
                nc.gpsimd.collective_compute(
                    "AllGather",
                    mybir.AluOpType.bypass,
                    replica_groups=[[0, 1, 2, 3], [4, 5, 6, 7]],
                    ins=[ckv_in.opt()],
                    outs=[ckv_gat.opt()],
                )
                for g in range(4):
                    nc.gpsimd.dma_start(ckv_nat[:, 4 * g : 4 * g + 4, :], ckv_gat[g])

                def rope_pair(raw_src_psum, dst, rc, jt, tag):
                    # raw_src_psum: [128,512] psum with 2 heads' raw rope rows.
                    raw = wka.tile([128, 512], BF16, tag=f"{tag}_raw", bufs=2)
                    nc.scalar.copy(raw, raw_src_psum)
                    psr = ps_b.tile([128, 512], F32, tag="ps_small")
                    nc.tensor.matmul(psr, lhsT=perm_s, rhs=raw, start=True, stop=True)
                    tmp = wka.tile([128, 512], BF16, tag=f"{tag}_cos", bufs=2)
                    nc.vector.tensor_mul(tmp, raw, cos_s[:, ts(jt, 512)])
                    tmp2 = wka.tile([128, 512], BF16, tag=f"{tag}_sin", bufs=2)
                    nc.vector.tensor_mul(tmp2, psr, sin_s[:, ts(jt, 512)])
                    nc.vector.tensor_add(dst[:, rc, ts(jt, 512)], tmp, tmp2)

                for jt in range(NJ):
                    xts = xs.tile([128, DC, 512], BF16)
                    xq_eng = nc.scalar if jt < 2 else nc.sync
                    for q4 in range(4):
                        xq_eng.dma_start(
                            xts[:, 4 * q4 : 4 * q4 + 4, :],
                            xT[:, 4 * q4 : 4 * q4 + 4, ts(jt, 512)],
                        )

                    # --- k_rope (transposed) + RoPE ---
                    for rc in range(2):
                        ps = ps_a.tile([128, 512], F32, tag="mm")
                        for dc in range(DC):
                            nc.tensor.matmul(
                                ps,
                                lhsT=wkrT_s[:, dc, ts(rc, 128)],
                                rhs=xts[:, dc, :],
                                start=(dc == 0),
                                stop=(dc == DC - 1),
                            )
                        rope_pair(ps, kTrot, rc, jt, "k")

                    # --- q content (transposed) ---
                    for fc in range(HPC):
                        ps = ps_a.tile([128, 512], F32, tag="mm")
                        for dc in range(DC):
                            nc.tensor.matmul(
                                ps,
                                lhsT=wqcT_s[:, dc, ts(fc, 128)],
                                rhs=xts[:, dc, :],
                                start=(dc == 0),
                                stop=(dc == DC - 1),
                            )
                        nc.vector.tensor_copy(qcT[:, fc, ts(jt, 512)], ps)

                    # --- q rope (transposed) + RoPE ---
                    for rc in range(2):
                        ps = ps_a.tile([128, 512], F32, tag="mm")
                        for dc in range(DC):
                            nc.tensor.matmul(
                                ps,
                                lhsT=wqrT_s[:, dc, ts(rc, 128)],
                                rhs=xts[:, dc, :],
                                start=(dc == 0),
                                stop=(dc == DC - 1),
                            )
                        rope_pair(ps, qTrot, rc, jt, "q")

            # ============== Phase B: attention =============================
            with (
                tc.tile_pool(name="attw", bufs=1) as attw,
                tc.tile_pool(name="qa", bufs=1) as qa_pool,
                tc.tile_pool(name="ctxp", bufs=1) as ctxp,
                tc.tile_pool(name="exps", bufs=10) as exps,
                tc.tile_pool(name="wkb", bufs=3) as wkb,
                tc.tile_pool(name="dnd", bufs=2, space="DRAM") as dnd,
            ):
                ctxT = ctxp.tile([128, HPC, T], BF16)      # [hd, h, t]
                wuk_s = attw.tile([128, HPC, LAT], BF16)
                nc.scalar.dma_start(wuk_s, wuk[:, :, :])
                wuvT_s = attw.tile([128, HPC, LC, HD], BF16)
                nc.scalar.dma_start(wuvT_s, wuvT[:, :, :, :])
                owT_s = attw.tile([128, HPC, D_OUT], BF16)
                for hc4 in range(HPC):
                    nc.scalar.dma_start(owT_s[:, hc4, :], owT[:, hc4, :])
                masks_s = attw.tile([128, 4, 512], BF16)
                nc.scalar.dma_start(masks_s, masks[:, :, :])
                ones16_s = attw.tile([128, 128], BF16)
                nc.scalar.dma_start(ones16_s, ones16[:, :])

                # build ckvT from the gathered ckv_nat with PE transposes
                # (the DMA-transpose path is ~1.4us per 128x128 tile and
                # would serialize for ~90us behind the AllGather)
                for tt in range(NT):
                    for lc2 in range(LC):
                        pst = ps_b.tile([128, 128], BF16, tag="ps_small")
                        nc.tensor.transpose(
                            pst, ckv_nat[:, tt, ts(lc2, 128)], ident_s
                        )
                        if (tt * LC + lc2) % 2 == 0:
                            nc.vector.tensor_copy(ckvT[:, lc2, ts(tt, 128)], pst)
                        else:
                            nc.scalar.copy(ckvT[:, lc2, ts(tt, 128)], pst)

                for h in range(HPC):
                    # absorbed q: qa = W_UK_h.T-contract with q_content
                    qaT = qa_pool.tile([128, LC, T], BF16, tag="qaT")
                    for lc in range(LC):
                        for jt in range(NJ):
                            ps = ps_b.tile([128, 512], F32, tag="ps_small")
                            nc.tensor.matmul(
                                ps,
                                lhsT=wuk_s[:, h, ts(lc, 128)],
                                rhs=qcT[:, h, ts(jt, 512)],
                                start=True,
                                stop=True,
                            )
                            if (lc * NJ + jt) % 2 == 0:
                                nc.vector.tensor_copy(qaT[:, lc, ts(jt, 512)], ps)
                            else:
                                nc.scalar.copy(qaT[:, lc, ts(jt, 512)], ps)

                    hb = (h % 2) * 64
                    rc = h // 2
                    for j in range(NJ):
                        ntk = 4 * (j + 1)
                        ps_ctx = ps_pv.tile([128, LC, 512], F32, tag="ps_ctx")
                        ps_dn = ps_b.tile([1, 512], F32, tag="ps_small")
                        def qk_block(tk):
                            # diagonal blocks: queries left of the block's
                            # first key are fully masked -- skip those columns
                            r = tk - 4 * j
                            q0 = 128 * r if r > 0 else 0
                            ps_s = ps_a.tile([128, 512], F32, tag="mm")
                            for lc in range(LC):
                                nc.tensor.matmul(
                                    ps_s[:, q0:],
                                    lhsT=ckvT[:, lc, ts(tk, 128)],
                                    rhs=qaT[:, lc, 512 * j + q0 : 512 * (j + 1)],
                                    start=(lc == 0),
                                    stop=False,
                                )
                            nc.tensor.matmul(
                                ps_s[:, q0:],
                                lhsT=kTrot[hb : hb + 64, rc, ts(tk, 128)],
                                rhs=qTrot[hb : hb + 64, rc, 512 * j + q0 : 512 * (j + 1)],
                                start=False,
                                stop=True,
                            )
                            ex = exps.tile([128, 512], BF16, tag="exp")
                            nc.scalar.activation(ex[:, q0:], ps_s[:, q0:], AF.Exp, scale=SCALE)
                            if r >= 0:
                                nc.gpsimd.tensor_mul(
                                    ex[:, q0:], ex[:, q0:], masks_s[:, r, q0:]
                                )
                            return ex, q0

                        def pv_block(tk, ex, q0):
                            for lc in range(LC):
                                nc.tensor.matmul(
                                    ps_ctx[:, lc, q0:],
                                    lhsT=ckv_nat[:, tk, ts(lc, 128)],
                                    rhs=ex[:, q0:],
                                    start=(tk == 0),
                                    stop=(tk == ntk - 1),
                                )
                            nc.tensor.matmul(
                                ps_dn[:, q0:],
                                lhsT=ones16_s[:, 0:1],
                                rhs=ex[:, q0:],
                                start=(tk == 0),
                                stop=(tk == ntk - 1),
                            )

                        # software pipeline: PV of pair p runs under QK of
                        # pair p+1, giving exp/mask a full QK-pair to finish
                        pend = None
                        for tk2 in range(ntk // 2):
                            ex0, q00 = qk_block(2 * tk2)
                            ex1, q01 = qk_block(2 * tk2 + 1)
                            if pend is not None:
                                pv_block(*pend[0])
                                pv_block(*pend[1])
                            pend = ((2 * tk2, ex0, q00), (2 * tk2 + 1, ex1, q01))
                        pv_block(*pend[0])
                        pv_block(*pend[1])
                        # reciprocal on the 1-partition row (fast-approx DVE
                        # op, ~18 bits; denominators are positive sums of
                        # exps), then broadcast via a DRAM round-trip
                        dn_r = wkb.tile([1, 512], F32, tag="dr")
                        nc.vector.reciprocal_approx_fast(out=dn_r, in_=ps_dn)
                        dr_d = dnd.tile([1, 512], F32, tag="dr_d")
                        nc.sync.dma_start(dr_d, dn_r)
                        db = wkb.tile([128, 512], F32, tag="db")
                        nc.gpsimd.dma_start(db, dr_d.to_broadcast((128, 512)))
                        # UV: ctxT_h = W_UV_h.T-contract with ctx_lat
                        cl = wkb.tile([128, LC, 512], BF16, tag="ctxlat")
                        for lc in range(LC):
                            if lc % 2 == 0:
                                nc.vector.tensor_copy(cl[:, lc, :], ps_ctx[:, lc, :])
                            else:
                                nc.scalar.copy(cl[:, lc, :], ps_ctx[:, lc, :])
                        ps_uv = ps_b.tile([128, 512], F32, tag="ps_small")
                        for lc in range(LC):
                            nc.tensor.matmul(
                                ps_uv,
                                lhsT=wuvT_s[:, h, lc, :],
                                rhs=cl[:, lc, :],
                                start=(lc == 0),
                                stop=(lc == LC - 1),
                            )
                        # drain PSUM fast with a plain copy; the denominator
                        # multiply happens off the critical path once db lands
                        uvr = wkb.tile([128, 512], BF16, tag="uvr")
                        nc.vector.tensor_copy(uvr, ps_uv)
                        nc.gpsimd.tensor_mul(ctxT[:, h, ts(j, 512)], uvr, db)

                # ============== Phase C: output projection =================
                with tc.tile_pool(name="outs", bufs=3) as outs:
                    for tt in range(NT):
                        ot = outs.tile([128, D_OUT], BF16, tag="ot")
                        for oc in range(D_OUT // 512):
                            ps = ps_a.tile([128, 512], F32, tag="mm")
                            for hc in range(HPC):
                                nc.tensor.matmul(
                                    ps,
                                    lhsT=ctxT[:, hc, ts(tt, 128)],
                                    rhs=owT_s[:, hc, ts(oc, 512)],
                                    start=(hc == 0),
                                    stop=(hc == HPC - 1),
                                )
                            if oc % 2 == 0:
                                nc.vector.tensor_copy(ot[:, ts(oc, 512)], ps)
                            else:
                                nc.scalar.copy(ot[:, ts(oc, 512)], ps)
                        nc.sync.dma_start(out_p[ts(tt, 128), :], ot)

    nc.finalize()
    return nc


def _part_major(a2d):
    """[R, C] -> [128, R//128, C] with partition = R % 128."""
    r, c = a2d.shape
    return np.ascontiguousarray(
        a2d.reshape(r // 128, 128, c).transpose(1, 0, 2)
    )


def make_in_maps(x, W_DKV, kv_norm_w, W_KR, W_Q, W_UK, W_UV, out_w, offset, T):
    """Host-side sharding/layout prep. Returns the 8 per-core input dicts."""
    f32 = np.float32
    x = np.asarray(x, f32)
    W_DKV = np.asarray(W_DKV, f32)
    kv_norm_w = np.asarray(kv_norm_w, f32)
    W_KR = np.asarray(W_KR, f32)
    W_Q = np.asarray(W_Q, f32)
    W_UK = np.asarray(W_UK, f32)
    W_UV = np.asarray(W_UV, f32)
    out_w = np.asarray(out_w, f32)
    offset = int(np.asarray(offset))

    def bf(a):
        return np.ascontiguousarray(a).astype(NPBF16)

    # rope tables, mirroring the reference's f32 arithmetic
    inv_freq = (1.0 / (THETA ** (np.arange(0, RD, 2, dtype=f32) / f32(RD)))).astype(f32)
    pos = np.arange(offset, offset + T, dtype=f32)
    ang = (pos[:, None] * inv_freq[None, :]).astype(f32)     # [T, RD/2]
    ang = np.concatenate([ang, ang], axis=-1)                # [T, RD]
    cos_t = np.cos(ang).T                                    # [RD, T]
    sin_t = np.sin(ang).T
    cosT = np.concatenate([cos_t, cos_t], 0)                 # [128, T]
    sinT = np.concatenate([sin_t, sin_t], 0)

    # signed rotate-half permutation (2 heads per 128 partitions), as lhsT
    M = np.zeros((RD, RD), f32)
    for i in range(RD // 2):
        M[i, i + RD // 2] = -1.0
        M[i + RD // 2, i] = 1.0
    perm128 = np.zeros((128, 128), f32)
    perm128[:64, :64] = M
    perm128[64:, 64:] = M
    perm_lhsT = perm128.T

    # diagonal causal masks: block r masked where (128 r + p) > f
    p_idx = np.arange(128)[:, None]
    f_idx = np.arange(512)[None, :]
    masks = np.stack(
        [(128 * r + p_idx <= f_idx).astype(f32) for r in range(4)], axis=1
    )  # [128, 4, 512]

    kvw = np.broadcast_to(kv_norm_w[None, :], (128, LAT)).astype(f32)
    ones16 = np.ones((128, 128), f32)
    ident128 = np.eye(128, dtype=f32)

    wuk_full = W_UK.reshape(H, HD, LAT)
    wuv_full = W_UV.reshape(H, HD, LAT)

    in_maps = []
    for b in range(2):
        xTb = bf(_part_major(x[b].T))  # [128, DC, T]
        for hg in range(4):
            hs = slice(HPC * hg * HD, HPC * (hg + 1) * HD)          # content rows
            rs = slice(D_OUT + HPC * hg * RD, D_OUT + HPC * (hg + 1) * RD)
            heads = slice(HPC * hg, HPC * (hg + 1))
            wuk_c = wuk_full[heads]                                  # [4,128,512]
            wuv_c = wuv_full[heads]
            in_maps.append(
                {
                    "xT": xTb,
                    "xq": np.ascontiguousarray(xTb[:, :, 512 * hg : 512 * (hg + 1)]),
                    "wdkvT": bf(_part_major(W_DKV.T)),
                    "wkrT": bf(_part_major(W_KR[HPC * hg * RD : HPC * (hg + 1) * RD].T)),
                    "wqcT": bf(_part_major(W_Q[hs].T)),
                    "wqrT": bf(_part_major(W_Q[rs].T)),
                    "wuk": bf(wuk_c.transpose(1, 0, 2)),             # [128,4,512]
                    "wuvT": bf(
                        wuv_c.transpose(0, 2, 1)                     # [4,512,128]
                        .reshape(HPC, LC, 128, HD)
                        .transpose(2, 0, 1, 3)                       # [128,4,4,128]
                    ),
                    "owT": bf(
                        out_w[:, hs].T.reshape(HPC, 128, D_OUT).transpose(1, 0, 2)
                    ),
                    "kvw": np.ascontiguousarray(kvw),
                    "cosT": bf(cosT),
                    "sinT": bf(sinT),
                    "perm": bf(perm_lhsT),
                    "masks": bf(masks),
                    "ones16": bf(ones16),
                    "ident": bf(ident128),
                }
            )
    return in_maps


_NC_CACHE = {}


def get_nc(T=2048):
    if T not in _NC_CACHE:
        _NC_CACHE[T] = build_mla_nc(T)
    return _NC_CACHE[T]


LAST_RESULTS = None


def kernel(x, W_DKV, kv_norm_w, W_KR, W_Q, W_UK, W_UV, out_w, out_b, offset):
    global LAST_RESULTS
    import os

    x = np.asarray(x, np.float32)
    B, T, _ = x.shape
    nc = get_nc(T)
    in_maps = make_in_maps(
        x, W_DKV, kv_norm_w, W_KR, W_Q, W_UK, W_UV, out_w, offset, T
    )
    trace = os.environ.get("MLA_TRACE", "0") == "1"
    res = run_bass_kernel_spmd(
        nc, in_maps, core_ids=list(range(8)), trace=trace
    )
    LAST_RESULTS = res
    out = np.zeros((B, T, D_OUT), np.float32)
    for c, r in enumerate(res.results):
        out[c // 4] += np.asarray(r["out_p"], np.float32)
    out += np.asarray(out_b, np.float32)[None, None, :]
    return out



# revision 26
# speedup vs baseline: 1.2896x; 1.1568x over previous
"""DeepSeekV3 latent attention (MLA) Trainium2 Bass kernel.

Sharding: 8 cores = 2 batches x 4 head-groups (4 heads each).
Each core computes, for its (batch b, head-group hg):
  - c_kv = RMSNorm(x_b @ W_DKV.T) * w        (replicated across the 4 hg cores)
  - k_rope / q projections for its 4 heads (weights sliced on the head axis)
  - causal latent attention (no-max-sub softmax, exp/sum form)
  - out_partial = ctx_hg @ out_w[:, hg_cols].T   (row-parallel partial)
Host sums the 4 partials per batch and adds the bias.

Device layout notes: everything is kept "feature-on-partition, token-on-free"
(transposed) so attention scores come out as S^T [tk, tq] and the probs feed
the PV matmul directly with no on-chip transposes of the big tensors.  The
softmax denominator is computed with a ones-column matmul; RoPE's rotate-half
is a signed 64x64 permutation matmul plus two multiplies.
"""

import numpy as np
import ml_dtypes

import concourse.bass as bass
import concourse.tile as tile
from concourse import bacc
from concourse import mybir
from concourse.bass import ts
from concourse.bass_utils import run_bass_kernel_spmd

BF16 = mybir.dt.bfloat16
F32 = mybir.dt.float32
NPBF16 = ml_dtypes.bfloat16

H, HD, RD, LAT = 16, 128, 64, 512
D_IN = 2048
D_OUT = H * HD
HPC = 4  # heads per core
LC = LAT // 128
EPS = 1e-6
THETA = 10000.0
SCALE = 1.0 / float(np.sqrt(np.float32(HD + RD)))
AF = mybir.ActivationFunctionType


def build_mla_nc(T=2048):
    nc = bacc.Bacc("TRN2", target_bir_lowering=False)
    DC = D_IN // 128      # 16 contraction chunks for the x projections
    NT = T // 128         # 128-token tiles
    NJ = T // 512         # 512-token query supertiles
    LC = LAT // 128       # 4 latent chunks

    # ---------------- I/O (all layouts are host-prepared, partition-major) ---
    xT = nc.dram_tensor("xT", [128, DC, T], BF16, kind="ExternalInput")
    wdkvT = nc.dram_tensor("wdkvT", [128, DC, LAT], BF16, kind="ExternalInput")
    wkrT = nc.dram_tensor("wkrT", [128, DC, HPC * RD], BF16, kind="ExternalInput")
    wqcT = nc.dram_tensor("wqcT", [128, DC, HPC * HD], BF16, kind="ExternalInput")
    wqrT = nc.dram_tensor("wqrT", [128, DC, HPC * RD], BF16, kind="ExternalInput")
    wuk = nc.dram_tensor("wuk", [128, HPC, LAT], BF16, kind="ExternalInput")
    wuvT = nc.dram_tensor("wuvT", [128, HPC, LC, HD], BF16, kind="ExternalInput")
    owT = nc.dram_tensor("owT", [128, HPC, D_OUT], BF16, kind="ExternalInput")
    kvw = nc.dram_tensor("kvw", [128, LAT], F32, kind="ExternalInput")
    cosT = nc.dram_tensor("cosT", [128, T], BF16, kind="ExternalInput")
    sinT = nc.dram_tensor("sinT", [128, T], BF16, kind="ExternalInput")
    perm = nc.dram_tensor("perm", [128, 128], BF16, kind="ExternalInput")
    masks = nc.dram_tensor("masks", [128, 4, 512], BF16, kind="ExternalInput")
    ones16 = nc.dram_tensor("ones16", [128, 128], BF16, kind="ExternalInput")
    out_p = nc.dram_tensor("out_p", [T, D_OUT], BF16, kind="ExternalOutput")

    with tile.TileContext(nc) as tc:
        with (
            tc.tile_pool(name="persist", bufs=1) as persist,
            tc.tile_pool(name="ps_a", bufs=2, space="PSUM") as ps_a,
            tc.tile_pool(name="ps_b", bufs=2, space="PSUM") as ps_b,
            tc.tile_pool(name="ps_pv", bufs=1, space="PSUM") as ps_pv,
        ):
            ckv_nat = persist.tile([128, NT, LAT], BF16)   # [t%128, ttile, lat]
            ckvT = persist.tile([128, LC, T], BF16)        # [lat%128, lc, t]
            kTrot = persist.tile([128, 2, T], BF16)        # [pairrow, h//2, t]
            qTrot = persist.tile([128, 2, T], BF16)
            qcT = persist.tile([128, HPC, T], BF16)        # [hd, h, t]
            ctxT = persist.tile([128, HPC, T], BF16)       # [hd, h, t]

            # ============== Phase A: projections + RMSNorm + RoPE ===========
            with (
                tc.tile_pool(name="projw", bufs=1) as projw,
                tc.tile_pool(name="xs", bufs=2) as xs,
                tc.tile_pool(name="wka", bufs=3) as wka,
            ):
                wdkvT_s = projw.tile([128, DC, LAT], BF16)
                wkrT_s = projw.tile([128, DC, HPC * RD], BF16)
                wqcT_s = projw.tile([128, DC, HPC * HD], BF16)
                wqrT_s = projw.tile([128, DC, HPC * RD], BF16)
                for dc in range(DC):
                    nc.sync.dma_start(wdkvT_s[:, dc, :], wdkvT[:, dc, :])
                for dc in range(DC):
                    nc.sync.dma_start(wkrT_s[:, dc, :], wkrT[:, dc, :])
                    nc.sync.dma_start(wqcT_s[:, dc, :], wqcT[:, dc, :])
                    nc.sync.dma_start(wqrT_s[:, dc, :], wqrT[:, dc, :])
                cos_s = projw.tile([128, T], BF16)
                nc.sync.dma_start(cos_s, cosT[:, :])
                sin_s = projw.tile([128, T], BF16)
                nc.sync.dma_start(sin_s, sinT[:, :])
                perm_s = projw.tile([128, 128], BF16)
                nc.sync.dma_start(perm_s, perm[:, :])
                kvw_s = projw.tile([128, LAT], F32)
                nc.sync.dma_start(kvw_s, kvw[:, :])
                eps_s = projw.tile([128, 1], F32)
                nc.vector.memset(eps_s, EPS)

                def rope_pair(raw_src_psum, dst, rc, jt, tag):
                    # raw_src_psum: [128,512] psum with 2 heads' raw rope rows.
                    raw = wka.tile([128, 512], BF16, tag=f"{tag}_raw")
                    nc.scalar.copy(raw, raw_src_psum)
                    psr = ps_b.tile([128, 512], F32, tag="ps_small")
                    nc.tensor.matmul(psr, lhsT=perm_s, rhs=raw, start=True, stop=True)
                    tmp = wka.tile([128, 512], BF16, tag=f"{tag}_cos")
                    nc.vector.tensor_mul(tmp, raw, cos_s[:, ts(jt, 512)])
                    tmp2 = wka.tile([128, 512], BF16, tag=f"{tag}_sin")
                    nc.vector.tensor_mul(tmp2, psr, sin_s[:, ts(jt, 512)])
                    nc.vector.tensor_add(dst[:, rc, ts(jt, 512)], tmp, tmp2)

                for jt in range(NJ):
                    xts = xs.tile([128, DC, 512], BF16)
                    if jt == 0:
                        # fine-grained so the first contraction chunk lands
                        # as early as possible at kernel start
                        for dcx in range(DC):
                            nc.scalar.dma_start(
                                xts[:, dcx, :], xT[:, dcx, ts(jt, 512)]
                            )
                    else:
                        for q4 in range(4):
                            nc.scalar.dma_start(
                                xts[:, 4 * q4 : 4 * q4 + 4, :],
                                xT[:, 4 * q4 : 4 * q4 + 4, ts(jt, 512)],
                            )

                    # --- c_kv (natural layout) + RMSNorm ---
                    for tt4 in range(4):
                        tt = jt * 4 + tt4
                        ps = ps_a.tile([128, 512], F32, tag="mm")
                        for dc in range(DC):
                            nc.tensor.matmul(
                                ps,
                                lhsT=xts[:, dc, ts(tt4, 128)],
                                rhs=wdkvT_s[:, dc, :],
                                start=(dc == 0),
                                stop=(dc == DC - 1),
                            )
                        sq = wka.tile([128, LAT], BF16, tag="sq")
                        ssum = wka.tile([128, 1], F32, tag="ssum")
                        nc.scalar.activation(sq, ps, AF.Square, accum_out=ssum)
                        rstd = wka.tile([128, 1], F32, tag="rstd")
                        nc.scalar.activation(
                            rstd, ssum, AF.Sqrt, bias=eps_s, scale=1.0 / LAT
                        )
                        nc.vector.reciprocal(rstd, rstd)
                        tmp = wka.tile([128, LAT], BF16, tag="ckvtmp")
                        nc.scalar.activation(tmp, ps, AF.Copy, scale=rstd)
                        nc.vector.tensor_mul(ckv_nat[:, tt, :], tmp, kvw_s)
                        # transposed copy for the QK side
                        for lc in range(LC):
                            nc.sync.dma_start(
                                ckvT[:, lc, ts(tt, 128)],
                                ckv_nat[:, tt, ts(lc, 128)],
                                transpose=True,
                            )

                    # --- k_rope (transposed) + RoPE ---
                    for rc in range(2):
                        ps = ps_a.tile([128, 512], F32, tag="mm")
                        for dc in range(DC):
                            nc.tensor.matmul(
                                ps,
                                lhsT=wkrT_s[:, dc, ts(rc, 128)],
                                rhs=xts[:, dc, :],
                                start=(dc == 0),
                                stop=(dc == DC - 1),
                            )
                        rope_pair(ps, kTrot, rc, jt, "k")

                    # --- q content (transposed) ---
                    for fc in range(HPC):
                        ps = ps_a.tile([128, 512], F32, tag="mm")
                        for dc in range(DC):
                            nc.tensor.matmul(
                                ps,
                                lhsT=wqcT_s[:, dc, ts(fc, 128)],
                                rhs=xts[:, dc, :],
                                start=(dc == 0),
                                stop=(dc == DC - 1),
                            )
                        nc.vector.tensor_copy(qcT[:, fc, ts(jt, 512)], ps)

                    # --- q rope (transposed) + RoPE ---
                    for rc in range(2):
                        ps = ps_a.tile([128, 512], F32, tag="mm")
                        for dc in range(DC):
                            nc.tensor.matmul(
                                ps,
                                lhsT=wqrT_s[:, dc, ts(rc, 128)],
                                rhs=xts[:, dc, :],
                                start=(dc == 0),
                                stop=(dc == DC - 1),
                            )
                        rope_pair(ps, qTrot, rc, jt, "q")

            # ============== Phase B: attention =============================
            with (
                tc.tile_pool(name="attw", bufs=1) as attw,
                tc.tile_pool(name="qa", bufs=1) as qa_pool,
                tc.tile_pool(name="exps", bufs=10) as exps,
                tc.tile_pool(name="wkb", bufs=3) as wkb,
                tc.tile_pool(name="dnd", bufs=2, space="DRAM") as dnd,
            ):
                wuk_s = attw.tile([128, HPC, LAT], BF16)
                nc.scalar.dma_start(wuk_s, wuk[:, :, :])
                wuvT_s = attw.tile([128, HPC, LC, HD], BF16)
                nc.scalar.dma_start(wuvT_s, wuvT[:, :, :, :])
                owT_s = attw.tile([128, HPC, D_OUT], BF16)
                for hc4 in range(HPC):
                    nc.scalar.dma_start(owT_s[:, hc4, :], owT[:, hc4, :])
                masks_s = attw.tile([128, 4, 512], BF16)
                nc.sync.dma_start(masks_s, masks[:, :, :])
                ones16_s = attw.tile([128, 128], BF16)
                nc.sync.dma_start(ones16_s, ones16[:, :])

                for h in range(HPC):
                    # absorbed q: qa = W_UK_h.T-contract with q_content
                    qaT = qa_pool.tile([128, LC, T], BF16, tag="qaT")
                    for lc in range(LC):
                        for jt in range(NJ):
                            ps = ps_b.tile([128, 512], F32, tag="ps_small")
                            nc.tensor.matmul(
                                ps,
                                lhsT=wuk_s[:, h, ts(lc, 128)],
                                rhs=qcT[:, h, ts(jt, 512)],
                                start=True,
                                stop=True,
                            )
                            if (lc * NJ + jt) % 2 == 0:
                                nc.vector.tensor_copy(qaT[:, lc, ts(jt, 512)], ps)
                            else:
                                nc.scalar.copy(qaT[:, lc, ts(jt, 512)], ps)

                    hb = (h % 2) * 64
                    rc = h // 2
                    for j in range(NJ):
                        ntk = 4 * (j + 1)
                        ps_ctx = ps_pv.tile([128, LC, 512], F32, tag="ps_ctx")
                        exsum = wkb.tile([128, 512], BF16, tag="exsum")
                        def qk_block(tk):
                            # diagonal blocks: queries left of the block's
                            # first key are fully masked -- skip those columns
                            r = tk - 4 * j
                            q0 = 128 * r if r > 0 else 0
                            ps_s = ps_a.tile([128, 512], F32, tag="mm")
                            for lc in range(LC):
                                nc.tensor.matmul(
                                    ps_s[:, q0:],
                                    lhsT=ckvT[:, lc, ts(tk, 128)],
                                    rhs=qaT[:, lc, 512 * j + q0 : 512 * (j + 1)],
                                    start=(lc == 0),
                                    stop=False,
                                )
                            nc.tensor.matmul(
                                ps_s[:, q0:],
                                lhsT=kTrot[hb : hb + 64, rc, ts(tk, 128)],
                                rhs=qTrot[hb : hb + 64, rc, 512 * j + q0 : 512 * (j + 1)],
                                start=False,
                                stop=True,
                            )
                            ex = exps.tile([128, 512], BF16, tag="exp")
                            nc.scalar.activation(ex[:, q0:], ps_s[:, q0:], AF.Exp, scale=SCALE)
                            if r >= 0:
                                nc.gpsimd.tensor_mul(
                                    ex[:, q0:], ex[:, q0:], masks_s[:, r, q0:]
                                )
                            # running key-block sum for the softmax
                            # denominator (idle gpsimd; replaces a per-block
                            # ones-column matmul on the tensor engine)
                            if tk == 0:
                                nc.gpsimd.tensor_copy(exsum, ex)
                            else:
                                nc.gpsimd.tensor_add(
                                    exsum[:, q0:], exsum[:, q0:], ex[:, q0:]
                                )
                            return ex, q0

                        def pv_block(tk, ex, q0):
                            for lc in range(LC):
                                nc.tensor.matmul(
                                    ps_ctx[:, lc, q0:],
                                    lhsT=ckv_nat[:, tk, ts(lc, 128)],
                                    rhs=ex[:, q0:],
                                    start=(tk == 0),
                                    stop=(tk == ntk - 1),
                                )

                        # software pipeline: PV of pair p runs under QK of
                        # pair p+1, giving exp/mask a full QK-pair to finish
                        pend = None
                        for tk2 in range(ntk // 2):
                            ex0, q00 = qk_block(2 * tk2)
                            ex1, q01 = qk_block(2 * tk2 + 1)
                            if pend is not None:
                                pv_block(*pend[0])
                                pv_block(*pend[1])
                            pend = ((2 * tk2, ex0, q00), (2 * tk2 + 1, ex1, q01))
                        pv_block(*pend[0])
                        pv_block(*pend[1])
                        # single cross-partition reduction of the accumulated
                        # exp sums gives the softmax denominator
                        ps_dn = ps_b.tile([1, 512], F32, tag="ps_small")
                        nc.tensor.matmul(
                            ps_dn,
                            lhsT=ones16_s[:, 0:1],
                            rhs=exsum,
                            start=True,
                            stop=True,
                        )
                        # reciprocal on the 1-partition row (fast-approx DVE
                        # op, ~18 bits; denominators are positive sums of
                        # exps), then broadcast via a DRAM round-trip
                        dn_r = wkb.tile([1, 512], F32, tag="dr")
                        nc.vector.reciprocal_approx_fast(out=dn_r, in_=ps_dn)
                        dr_d = dnd.tile([1, 512], F32, tag="dr_d")
                        nc.sync.dma_start(dr_d, dn_r)
                        db = wkb.tile([128, 512], F32, tag="db")
                        nc.gpsimd.dma_start(db, dr_d.to_broadcast((128, 512)))
                        # UV: ctxT_h = W_UV_h.T-contract with ctx_lat
                        cl = wkb.tile([128, LC, 512], BF16, tag="ctxlat")
                        for lc in range(LC):
                            if lc % 2 == 0:
                                nc.vector.tensor_copy(cl[:, lc, :], ps_ctx[:, lc, :])
                            else:
                                nc.scalar.copy(cl[:, lc, :], ps_ctx[:, lc, :])
                        ps_uv = ps_b.tile([128, 512], F32, tag="ps_small")
                        for lc in range(LC):
                            nc.tensor.matmul(
                                ps_uv,
                                lhsT=wuvT_s[:, h, lc, :],
                                rhs=cl[:, lc, :],
                                start=(lc == 0),
                                stop=(lc == LC - 1),
                            )
                        # drain PSUM fast with a plain copy; the denominator
                        # multiply happens off the critical path once db lands
                        uvr = wkb.tile([128, 512], BF16, tag="uvr")
                        nc.vector.tensor_copy(uvr, ps_uv)
                        nc.gpsimd.tensor_mul(ctxT[:, h, ts(j, 512)], uvr, db)

                # ============== Phase C: output projection =================
                with tc.tile_pool(name="outs", bufs=3) as outs:
                    for tt in range(NT):
                        ot = outs.tile([128, D_OUT], BF16, tag="ot")
                        for oc in range(D_OUT // 512):
                            ps = ps_a.tile([128, 512], F32, tag="mm")
                            for hc in range(HPC):
                                nc.tensor.matmul(
                                    ps,
                                    lhsT=ctxT[:, hc, ts(tt, 128)],
                                    rhs=owT_s[:, hc, ts(oc, 512)],
                                    start=(hc == 0),
                                    stop=(hc == HPC - 1),
                                )
                            if oc % 2 == 0:
                                nc.vector.tensor_copy(ot[:, ts(oc, 512)], ps)
                            else:
                                nc.scalar.copy(ot[:, ts(oc, 512)], ps)
                        nc.sync.dma_start(out_p[ts(tt, 128), :], ot)

    nc.finalize()
    return nc


def _part_major(a2d):
    """[R, C] -> [128, R//128, C] with partition = R % 128."""
    r, c = a2d.shape
    return np.ascontiguousarray(
        a2d.reshape(r // 128, 128, c).transpose(1, 0, 2)
    )


def make_in_maps(x, W_DKV, kv_norm_w, W_KR, W_Q, W_UK, W_UV, out_w, offset, T):
    """Host-side sharding/layout prep. Returns the 8 per-core input dicts."""
    f32 = np.float32
    x = np.asarray(x, f32)
    W_DKV = np.asarray(W_DKV, f32)
    kv_norm_w = np.asarray(kv_norm_w, f32)
    W_KR = np.asarray(W_KR, f32)
    W_Q = np.asarray(W_Q, f32)
    W_UK = np.asarray(W_UK, f32)
    W_UV = np.asarray(W_UV, f32)
    out_w = np.asarray(out_w, f32)
    offset = int(np.asarray(offset))

    def bf(a):
        return np.ascontiguousarray(a).astype(NPBF16)

    # rope tables, mirroring the reference's f32 arithmetic
    inv_freq = (1.0 / (THETA ** (np.arange(0, RD, 2, dtype=f32) / f32(RD)))).astype(f32)
    pos = np.arange(offset, offset + T, dtype=f32)
    ang = (pos[:, None] * inv_freq[None, :]).astype(f32)     # [T, RD/2]
    ang = np.concatenate([ang, ang], axis=-1)                # [T, RD]
    cos_t = np.cos(ang).T                                    # [RD, T]
    sin_t = np.sin(ang).T
    cosT = np.concatenate([cos_t, cos_t], 0)                 # [128, T]
    sinT = np.concatenate([sin_t, sin_t], 0)

    # signed rotate-half permutation (2 heads per 128 partitions), as lhsT
    M = np.zeros((RD, RD), f32)
    for i in range(RD // 2):
        M[i, i + RD // 2] = -1.0
        M[i + RD // 2, i] = 1.0
    perm128 = np.zeros((128, 128), f32)
    perm128[:64, :64] = M
    perm128[64:, 64:] = M
    perm_lhsT = perm128.T

    # diagonal causal masks: block r masked where (128 r + p) > f
    p_idx = np.arange(128)[:, None]
    f_idx = np.arange(512)[None, :]
    masks = np.stack(
        [(128 * r + p_idx <= f_idx).astype(f32) for r in range(4)], axis=1
    )  # [128, 4, 512]

    kvw = np.broadcast_to(kv_norm_w[None, :], (128, LAT)).astype(f32)
    ones16 = np.ones((128, 128), f32)

    wuk_full = W_UK.reshape(H, HD, LAT)
    wuv_full = W_UV.reshape(H, HD, LAT)

    in_maps = []
    for b in range(2):
        xTb = bf(_part_major(x[b].T))  # [128, DC, T]
        for hg in range(4):
            hs = slice(HPC * hg * HD, HPC * (hg + 1) * HD)          # content rows
            rs = slice(D_OUT + HPC * hg * RD, D_OUT + HPC * (hg + 1) * RD)
            heads = slice(HPC * hg, HPC * (hg + 1))
            wuk_c = wuk_full[heads]                                  # [4,128,512]
            wuv_c = wuv_full[heads]
            in_maps.append(
                {
                    "xT": xTb,
                    "wdkvT": bf(_part_major(W_DKV.T)),
                    "wkrT": bf(_part_major(W_KR[HPC * hg * RD : HPC * (hg + 1) * RD].T)),
                    "wqcT": bf(_part_major(W_Q[hs].T)),
                    "wqrT": bf(_part_major(W_Q[rs].T)),
                    "wuk": bf(wuk_c.transpose(1, 0, 2)),             # [128,4,512]
                    "wuvT": bf(
                        wuv_c.transpose(0, 2, 1)                     # [4,512,128]
                        .reshape(HPC, LC, 128, HD)
                        .transpose(2, 0, 1, 3)                       # [128,4,4,128]
                    ),
                    "owT": bf(
                        out_w[:, hs].T.reshape(HPC, 128, D_OUT).transpose(1, 0, 2)
                    ),
                    "kvw": np.ascontiguousarray(kvw),
                    "cosT": bf(cosT),
                    "sinT": bf(sinT),
                    "perm": bf(perm_lhsT),
                    "masks": bf(masks),
                    "ones16": bf(ones16),
                }
            )
    return in_maps


_NC_CACHE = {}


def get_nc(T=2048):
    if T not in _NC_CACHE:
        _NC_CACHE[T] = build_mla_nc(T)
    return _NC_CACHE[T]


LAST_RESULTS = None


def kernel(x, W_DKV, kv_norm_w, W_KR, W_Q, W_UK, W_UV, out_w, out_b, offset):
    global LAST_RESULTS
    import os

    x = np.asarray(x, np.float32)
    B, T, _ = x.shape
    nc = get_nc(T)
    in_maps = make_in_maps(
        x, W_DKV, kv_norm_w, W_KR, W_Q, W_UK, W_UV, out_w, offset, T
    )
    trace = os.environ.get("MLA_TRACE", "0") == "1"
    res = run_bass_kernel_spmd(
        nc, in_maps, core_ids=list(range(8)), trace=trace
    )
    LAST_RESULTS = res
    out = np.zeros((B, T, D_OUT), np.float32)
    for c, r in enumerate(res.results):
        out[c // 4] += np.asarray(r["out_p"], np.float32)
    out += np.asarray(out_b, np.float32)[None, None, :]
    return out



# revision 27
# speedup vs baseline: 1.3401x; 1.0392x over previous
"""DeepSeekV3 latent attention (MLA) Trainium2 Bass kernel.

Sharding: 8 cores = 2 batches x 4 head-groups (4 heads each).
Each core computes, for its (batch b, head-group hg):
  - c_kv = RMSNorm(x_b @ W_DKV.T) * w        (replicated across the 4 hg cores)
  - k_rope / q projections for its 4 heads (weights sliced on the head axis)
  - causal latent attention (no-max-sub softmax, exp/sum form)
  - out_partial = ctx_hg @ out_w[:, hg_cols].T   (row-parallel partial)
Host sums the 4 partials per batch and adds the bias.

Device layout notes: everything is kept "feature-on-partition, token-on-free"
(transposed) so attention scores come out as S^T [tk, tq] and the probs feed
the PV matmul directly with no on-chip transposes of the big tensors.  The
softmax denominator is computed with a ones-column matmul; RoPE's rotate-half
is a signed 64x64 permutation matmul plus two multiplies.
"""

import numpy as np
import ml_dtypes

import concourse.bass as bass
import concourse.tile as tile
from concourse import bacc
from concourse import mybir
from concourse.bass import ts
from concourse.bass_utils import run_bass_kernel_spmd

BF16 = mybir.dt.bfloat16
F32 = mybir.dt.float32
NPBF16 = ml_dtypes.bfloat16

H, HD, RD, LAT = 16, 128, 64, 512
D_IN = 2048
D_OUT = H * HD
HPC = 4  # heads per core
LC = LAT // 128
EPS = 1e-6
THETA = 10000.0
SCALE = 1.0 / float(np.sqrt(np.float32(HD + RD)))
AF = mybir.ActivationFunctionType


def build_mla_nc(T=2048):
    nc = bacc.Bacc("TRN2", target_bir_lowering=False)
    DC = D_IN // 128      # 16 contraction chunks for the x projections
    NT = T // 128         # 128-token tiles
    NJ = T // 512         # 512-token query supertiles
    LC = LAT // 128       # 4 latent chunks

    # ---------------- I/O (all layouts are host-prepared, partition-major) ---
    xT = nc.dram_tensor("xT", [128, DC, T], BF16, kind="ExternalInput")
    wdkvT = nc.dram_tensor("wdkvT", [128, DC, LAT], BF16, kind="ExternalInput")
    wkrT = nc.dram_tensor("wkrT", [128, DC, HPC * RD], BF16, kind="ExternalInput")
    wqcT = nc.dram_tensor("wqcT", [128, DC, HPC * HD], BF16, kind="ExternalInput")
    wqrT = nc.dram_tensor("wqrT", [128, DC, HPC * RD], BF16, kind="ExternalInput")
    wuk = nc.dram_tensor("wuk", [128, HPC, LAT], BF16, kind="ExternalInput")
    wuvT = nc.dram_tensor("wuvT", [128, HPC, LC, HD], BF16, kind="ExternalInput")
    owT = nc.dram_tensor("owT", [128, HPC, D_OUT], BF16, kind="ExternalInput")
    kvw = nc.dram_tensor("kvw", [128, LAT], F32, kind="ExternalInput")
    cosT = nc.dram_tensor("cosT", [128, T], BF16, kind="ExternalInput")
    sinT = nc.dram_tensor("sinT", [128, T], BF16, kind="ExternalInput")
    perm = nc.dram_tensor("perm", [128, 128], BF16, kind="ExternalInput")
    masks = nc.dram_tensor("masks", [128, 4, 512], BF16, kind="ExternalInput")
    ones16 = nc.dram_tensor("ones16", [128, 128], BF16, kind="ExternalInput")
    out_p = nc.dram_tensor("out_p", [T, D_OUT], BF16, kind="ExternalOutput")

    with tile.TileContext(nc) as tc:
        with (
            tc.tile_pool(name="persist", bufs=1) as persist,
            tc.tile_pool(name="ps_a", bufs=2, space="PSUM") as ps_a,
            tc.tile_pool(name="ps_b", bufs=2, space="PSUM") as ps_b,
            tc.tile_pool(name="ps_pv", bufs=1, space="PSUM") as ps_pv,
        ):
            ckv_nat = persist.tile([128, NT, LAT], BF16)   # [t%128, ttile, lat]
            ckvT = persist.tile([128, LC, T], BF16)        # [lat%128, lc, t]
            kTrot = persist.tile([128, 2, T], BF16)        # [pairrow, h//2, t]
            qTrot = persist.tile([128, 2, T], BF16)
            qcT = persist.tile([128, HPC, T], BF16)        # [hd, h, t]
            ctxT = persist.tile([128, HPC, T], BF16)       # [hd, h, t]

            # ============== Phase A: projections + RMSNorm + RoPE ===========
            with (
                tc.tile_pool(name="projw", bufs=1) as projw,
                tc.tile_pool(name="xs", bufs=2) as xs,
                tc.tile_pool(name="wka", bufs=3) as wka,
            ):
                wdkvT_s = projw.tile([128, DC, LAT], BF16)
                wkrT_s = projw.tile([128, DC, HPC * RD], BF16)
                wqcT_s = projw.tile([128, DC, HPC * HD], BF16)
                wqrT_s = projw.tile([128, DC, HPC * RD], BF16)
                for dc in range(DC):
                    nc.sync.dma_start(wdkvT_s[:, dc, :], wdkvT[:, dc, :])
                for dc in range(DC):
                    nc.sync.dma_start(wkrT_s[:, dc, :], wkrT[:, dc, :])
                    nc.sync.dma_start(wqcT_s[:, dc, :], wqcT[:, dc, :])
                    nc.sync.dma_start(wqrT_s[:, dc, :], wqrT[:, dc, :])
                cos_s = projw.tile([128, T], BF16)
                nc.sync.dma_start(cos_s, cosT[:, :])
                sin_s = projw.tile([128, T], BF16)
                nc.sync.dma_start(sin_s, sinT[:, :])
                perm_s = projw.tile([128, 128], BF16)
                nc.sync.dma_start(perm_s, perm[:, :])
                kvw_s = projw.tile([128, LAT], F32)
                nc.sync.dma_start(kvw_s, kvw[:, :])
                eps_s = projw.tile([128, 1], F32)
                nc.vector.memset(eps_s, EPS)

                def rope_pair(raw_src_psum, dst, rc, jt, tag):
                    # raw_src_psum: [128,512] psum with 2 heads' raw rope rows.
                    raw = wka.tile([128, 512], BF16, tag=f"{tag}_raw")
                    nc.scalar.copy(raw, raw_src_psum)
                    psr = ps_b.tile([128, 512], F32, tag="ps_small")
                    nc.tensor.matmul(psr, lhsT=perm_s, rhs=raw, start=True, stop=True)
                    tmp = wka.tile([128, 512], BF16, tag=f"{tag}_cos")
                    nc.vector.tensor_mul(tmp, raw, cos_s[:, ts(jt, 512)])
                    tmp2 = wka.tile([128, 512], BF16, tag=f"{tag}_sin")
                    nc.vector.tensor_mul(tmp2, psr, sin_s[:, ts(jt, 512)])
                    nc.vector.tensor_add(dst[:, rc, ts(jt, 512)], tmp, tmp2)

                for jt in range(NJ):
                    xts = xs.tile([128, DC, 512], BF16)
                    if jt == 0:
                        # fine-grained so the first contraction chunk lands
                        # as early as possible at kernel start
                        for dcx in range(DC):
                            nc.scalar.dma_start(
                                xts[:, dcx, :], xT[:, dcx, ts(jt, 512)]
                            )
                    else:
                        for q4 in range(4):
                            nc.scalar.dma_start(
                                xts[:, 4 * q4 : 4 * q4 + 4, :],
                                xT[:, 4 * q4 : 4 * q4 + 4, ts(jt, 512)],
                            )

                    # --- c_kv (natural layout) + RMSNorm ---
                    for tt4 in range(4):
                        tt = jt * 4 + tt4
                        ps = ps_a.tile([128, 512], F32, tag="mm")
                        for dc in range(DC):
                            nc.tensor.matmul(
                                ps,
                                lhsT=xts[:, dc, ts(tt4, 128)],
                                rhs=wdkvT_s[:, dc, :],
                                start=(dc == 0),
                                stop=(dc == DC - 1),
                            )
                        sq = wka.tile([128, LAT], BF16, tag="sq")
                        ssum = wka.tile([128, 1], F32, tag="ssum")
                        nc.scalar.activation(sq, ps, AF.Square, accum_out=ssum)
                        rstd = wka.tile([128, 1], F32, tag="rstd")
                        nc.scalar.activation(
                            rstd, ssum, AF.Sqrt, bias=eps_s, scale=1.0 / LAT
                        )
                        nc.vector.reciprocal(rstd, rstd)
                        tmp = wka.tile([128, LAT], BF16, tag="ckvtmp")
                        nc.scalar.activation(tmp, ps, AF.Copy, scale=rstd)
                        nc.vector.tensor_mul(ckv_nat[:, tt, :], tmp, kvw_s)
                        # transposed copy for the QK side
                        for lc in range(LC):
                            nc.sync.dma_start(
                                ckvT[:, lc, ts(tt, 128)],
                                ckv_nat[:, tt, ts(lc, 128)],
                                transpose=True,
                            )

                    # --- k_rope (transposed) + RoPE ---
                    for rc in range(2):
                        ps = ps_a.tile([128, 512], F32, tag="mm")
                        for dc in range(DC):
                            nc.tensor.matmul(
                                ps,
                                lhsT=wkrT_s[:, dc, ts(rc, 128)],
                                rhs=xts[:, dc, :],
                                start=(dc == 0),
                                stop=(dc == DC - 1),
                            )
                        rope_pair(ps, kTrot, rc, jt, "k")

                    # --- q content (transposed) ---
                    for fc in range(HPC):
                        ps = ps_a.tile([128, 512], F32, tag="mm")
                        for dc in range(DC):
                            nc.tensor.matmul(
                                ps,
                                lhsT=wqcT_s[:, dc, ts(fc, 128)],
                                rhs=xts[:, dc, :],
                                start=(dc == 0),
                                stop=(dc == DC - 1),
                            )
                        nc.vector.tensor_copy(qcT[:, fc, ts(jt, 512)], ps)

                    # --- q rope (transposed) + RoPE ---
                    for rc in range(2):
                        ps = ps_a.tile([128, 512], F32, tag="mm")
                        for dc in range(DC):
                            nc.tensor.matmul(
                                ps,
                                lhsT=wqrT_s[:, dc, ts(rc, 128)],
                                rhs=xts[:, dc, :],
                                start=(dc == 0),
                                stop=(dc == DC - 1),
                            )
                        rope_pair(ps, qTrot, rc, jt, "q")

            # ============== Phase B: attention =============================
            with (
                tc.tile_pool(name="attw", bufs=1) as attw,
                tc.tile_pool(name="qa", bufs=1) as qa_pool,
                tc.tile_pool(name="exps", bufs=10) as exps,
                tc.tile_pool(name="wkb", bufs=3) as wkb,
                tc.tile_pool(name="dnd", bufs=2, space="DRAM") as dnd,
            ):
                wuk_s = attw.tile([128, HPC, LAT], BF16)
                nc.scalar.dma_start(wuk_s, wuk[:, :, :])
                wuvT_s = attw.tile([128, HPC, LC, HD], BF16)
                nc.scalar.dma_start(wuvT_s, wuvT[:, :, :, :])
                owT_s = attw.tile([128, HPC, D_OUT], BF16)
                for hc4 in range(HPC):
                    nc.scalar.dma_start(owT_s[:, hc4, :], owT[:, hc4, :])
                masks_s = attw.tile([128, 4, 512], BF16)
                nc.sync.dma_start(masks_s, masks[:, :, :])
                ones16_s = attw.tile([128, 128], BF16)
                nc.sync.dma_start(ones16_s, ones16[:, :])

                for h in range(HPC):
                    # absorbed q: qa = W_UK_h.T-contract with q_content
                    qaT = qa_pool.tile([128, LC, T], BF16, tag="qaT")
                    for lc in range(LC):
                        for jt in range(NJ):
                            ps = ps_b.tile([128, 512], F32, tag="ps_small")
                            nc.tensor.matmul(
                                ps,
                                lhsT=wuk_s[:, h, ts(lc, 128)],
                                rhs=qcT[:, h, ts(jt, 512)],
                                start=True,
                                stop=True,
                            )
                            if (lc * NJ + jt) % 2 == 0:
                                nc.vector.tensor_copy(qaT[:, lc, ts(jt, 512)], ps)
                            else:
                                nc.scalar.copy(qaT[:, lc, ts(jt, 512)], ps)

                    hb = (h % 2) * 64
                    rc = h // 2
                    for j in range(NJ):
                        ntk = 4 * (j + 1)
                        ps_ctx = ps_pv.tile([128, LC, 512], F32, tag="ps_ctx")
                        exsum = wkb.tile([128, 512], BF16, tag="exsum")
                        def qk_block(tk):
                            # diagonal blocks: queries left of the block's
                            # first key are fully masked -- skip those columns
                            r = tk - 4 * j
                            q0 = 128 * r if r > 0 else 0
                            ps_s = ps_a.tile([128, 512], F32, tag="mm")
                            for lc in range(LC):
                                nc.tensor.matmul(
                                    ps_s[:, q0:],
                                    lhsT=ckvT[:, lc, ts(tk, 128)],
                                    rhs=qaT[:, lc, 512 * j + q0 : 512 * (j + 1)],
                                    start=(lc == 0),
                                    stop=False,
                                )
                            nc.tensor.matmul(
                                ps_s[:, q0:],
                                lhsT=kTrot[hb : hb + 64, rc, ts(tk, 128)],
                                rhs=qTrot[hb : hb + 64, rc, 512 * j + q0 : 512 * (j + 1)],
                                start=False,
                                stop=True,
                            )
                            ex = exps.tile([128, 512], BF16, tag="exp")
                            nc.scalar.activation(ex[:, q0:], ps_s[:, q0:], AF.Exp, scale=SCALE)
                            if r >= 0:
                                nc.gpsimd.tensor_mul(
                                    ex[:, q0:], ex[:, q0:], masks_s[:, r, q0:]
                                )
                            # running key-block sum for the softmax
                            # denominator (idle gpsimd; replaces a per-block
                            # ones-column matmul on the tensor engine)
                            if tk == 0:
                                nc.vector.tensor_copy(exsum, ex)
                            else:
                                nc.vector.tensor_add(
                                    exsum[:, q0:], exsum[:, q0:], ex[:, q0:]
                                )
                            return ex, q0

                        def pv_block(tk, ex, q0):
                            for lc in range(LC):
                                nc.tensor.matmul(
                                    ps_ctx[:, lc, q0:],
                                    lhsT=ckv_nat[:, tk, ts(lc, 128)],
                                    rhs=ex[:, q0:],
                                    start=(tk == 0),
                                    stop=(tk == ntk - 1),
                                )

                        # software pipeline: PV of pair p runs under QK of
                        # pair p+1, giving exp/mask a full QK-pair to finish
                        pend = None
                        for tk2 in range(ntk // 2):
                            ex0, q00 = qk_block(2 * tk2)
                            ex1, q01 = qk_block(2 * tk2 + 1)
                            if pend is not None:
                                pv_block(*pend[0])
                                pv_block(*pend[1])
                            pend = ((2 * tk2, ex0, q00), (2 * tk2 + 1, ex1, q01))
                        pv_block(*pend[0])
                        pv_block(*pend[1])
                        # single cross-partition reduction of the accumulated
                        # exp sums gives the softmax denominator
                        ps_dn = ps_b.tile([1, 512], F32, tag="ps_small")
                        nc.tensor.matmul(
                            ps_dn,
                            lhsT=ones16_s[:, 0:1],
                            rhs=exsum,
                            start=True,
                            stop=True,
                        )
                        # reciprocal on the 1-partition row (fast-approx DVE
                        # op, ~18 bits; denominators are positive sums of
                        # exps), then broadcast via a DRAM round-trip
                        dn_r = wkb.tile([1, 512], F32, tag="dr")
                        nc.vector.reciprocal_approx_fast(out=dn_r, in_=ps_dn)
                        dr_d = dnd.tile([1, 512], F32, tag="dr_d")
                        nc.sync.dma_start(dr_d, dn_r)
                        db = wkb.tile([128, 512], F32, tag="db")
                        nc.gpsimd.dma_start(db, dr_d.to_broadcast((128, 512)))
                        # UV: ctxT_h = W_UV_h.T-contract with ctx_lat
                        cl = wkb.tile([128, LC, 512], BF16, tag="ctxlat")
                        for lc in range(LC):
                            if lc % 2 == 0:
                                nc.vector.tensor_copy(cl[:, lc, :], ps_ctx[:, lc, :])
                            else:
                                nc.scalar.copy(cl[:, lc, :], ps_ctx[:, lc, :])
                        ps_uv = ps_b.tile([128, 512], F32, tag="ps_small")
                        for lc in range(LC):
                            nc.tensor.matmul(
                                ps_uv,
                                lhsT=wuvT_s[:, h, lc, :],
                                rhs=cl[:, lc, :],
                                start=(lc == 0),
                                stop=(lc == LC - 1),
                            )
                        # drain PSUM fast with a plain copy; the denominator
                        # multiply happens off the critical path once db lands
                        uvr = wkb.tile([128, 512], BF16, tag="uvr")
                        nc.vector.tensor_copy(uvr, ps_uv)
                        nc.gpsimd.tensor_mul(ctxT[:, h, ts(j, 512)], uvr, db)

                # ============== Phase C: output projection =================
                with tc.tile_pool(name="outs", bufs=3) as outs:
                    for tt in range(NT):
                        ot = outs.tile([128, D_OUT], BF16, tag="ot")
                        for oc in range(D_OUT // 512):
                            ps = ps_a.tile([128, 512], F32, tag="mm")
                            for hc in range(HPC):
                                nc.tensor.matmul(
                                    ps,
                                    lhsT=ctxT[:, hc, ts(tt, 128)],
                                    rhs=owT_s[:, hc, ts(oc, 512)],
                                    start=(hc == 0),
                                    stop=(hc == HPC - 1),
                                )
                            if oc % 2 == 0:
                                nc.vector.tensor_copy(ot[:, ts(oc, 512)], ps)
                            else:
                                nc.scalar.copy(ot[:, ts(oc, 512)], ps)
                        nc.sync.dma_start(out_p[ts(tt, 128), :], ot)

    nc.finalize()
    return nc


def _part_major(a2d):
    """[R, C] -> [128, R//128, C] with partition = R % 128."""
    r, c = a2d.shape
    return np.ascontiguousarray(
        a2d.reshape(r // 128, 128, c).transpose(1, 0, 2)
    )


def make_in_maps(x, W_DKV, kv_norm_w, W_KR, W_Q, W_UK, W_UV, out_w, offset, T):
    """Host-side sharding/layout prep. Returns the 8 per-core input dicts."""
    f32 = np.float32
    x = np.asarray(x, f32)
    W_DKV = np.asarray(W_DKV, f32)
    kv_norm_w = np.asarray(kv_norm_w, f32)
    W_KR = np.asarray(W_KR, f32)
    W_Q = np.asarray(W_Q, f32)
    W_UK = np.asarray(W_UK, f32)
    W_UV = np.asarray(W_UV, f32)
    out_w = np.asarray(out_w, f32)
    offset = int(np.asarray(offset))

    def bf(a):
        return np.ascontiguousarray(a).astype(NPBF16)

    # rope tables, mirroring the reference's f32 arithmetic
    inv_freq = (1.0 / (THETA ** (np.arange(0, RD, 2, dtype=f32) / f32(RD)))).astype(f32)
    pos = np.arange(offset, offset + T, dtype=f32)
    ang = (pos[:, None] * inv_freq[None, :]).astype(f32)     # [T, RD/2]
    ang = np.concatenate([ang, ang], axis=-1)                # [T, RD]
    cos_t = np.cos(ang).T                                    # [RD, T]
    sin_t = np.sin(ang).T
    cosT = np.concatenate([cos_t, cos_t], 0)                 # [128, T]
    sinT = np.concatenate([sin_t, sin_t], 0)

    # signed rotate-half permutation (2 heads per 128 partitions), as lhsT
    M = np.zeros((RD, RD), f32)
    for i in range(RD // 2):
        M[i, i + RD // 2] = -1.0
        M[i + RD // 2, i] = 1.0
    perm128 = np.zeros((128, 128), f32)
    perm128[:64, :64] = M
    perm128[64:, 64:] = M
    perm_lhsT = perm128.T

    # diagonal causal masks: block r masked where (128 r + p) > f
    p_idx = np.arange(128)[:, None]
    f_idx = np.arange(512)[None, :]
    masks = np.stack(
        [(128 * r + p_idx <= f_idx).astype(f32) for r in range(4)], axis=1
    )  # [128, 4, 512]

    kvw = np.broadcast_to(kv_norm_w[None, :], (128, LAT)).astype(f32)
    ones16 = np.ones((128, 128), f32)

    wuk_full = W_UK.reshape(H, HD, LAT)
    wuv_full = W_UV.reshape(H, HD, LAT)

    in_maps = []
    for b in range(2):
        xTb = bf(_part_major(x[b].T))  # [128, DC, T]
        for hg in range(4):
            hs = slice(HPC * hg * HD, HPC * (hg + 1) * HD)          # content rows
            rs = slice(D_OUT + HPC * hg * RD, D_OUT + HPC * (hg + 1) * RD)
            heads = slice(HPC * hg, HPC * (hg + 1))
            wuk_c = wuk_full[heads]                                  # [4,128,512]
            wuv_c = wuv_full[heads]
            in_maps.append(
                {
                    "xT": xTb,
                    "wdkvT": bf(_part_major(W_DKV.T)),
                    "wkrT": bf(_part_major(W_KR[HPC * hg * RD : HPC * (hg + 1) * RD].T)),
                    "wqcT": bf(_part_major(W_Q[hs].T)),
                    "wqrT": bf(_part_major(W_Q[rs].T)),
                    "wuk": bf(wuk_c.transpose(1, 0, 2)),             # [128,4,512]
                    "wuvT": bf(
                        wuv_c.transpose(0, 2, 1)                     # [4,512,128]
                        .reshape(HPC, LC, 128, HD)
                        .transpose(2, 0, 1, 3)                       # [128,4,4,128]
                    ),
                    "owT": bf(
                        out_w[:, hs].T.reshape(HPC, 128, D_OUT).transpose(1, 0, 2)
                    ),
                    "kvw": np.ascontiguousarray(kvw),
                    "cosT": bf(cosT),
                    "sinT": bf(sinT),
                    "perm": bf(perm_lhsT),
                    "masks": bf(masks),
                    "ones16": bf(ones16),
                }
            )
    return in_maps


_NC_CACHE = {}


def get_nc(T=2048):
    if T not in _NC_CACHE:
        _NC_CACHE[T] = build_mla_nc(T)
    return _NC_CACHE[T]


LAST_RESULTS = None


def kernel(x, W_DKV, kv_norm_w, W_KR, W_Q, W_UK, W_UV, out_w, out_b, offset):
    global LAST_RESULTS
    import os

    x = np.asarray(x, np.float32)
    B, T, _ = x.shape
    nc = get_nc(T)
    in_maps = make_in_maps(
        x, W_DKV, kv_norm_w, W_KR, W_Q, W_UK, W_UV, out_w, offset, T
    )
    trace = os.environ.get("MLA_TRACE", "0") == "1"
    res = run_bass_kernel_spmd(
        nc, in_maps, core_ids=list(range(8)), trace=trace
    )
    LAST_RESULTS = res
    out = np.zeros((B, T, D_OUT), np.float32)
    for c, r in enumerate(res.results):
        out[c // 4] += np.asarray(r["out_p"], np.float32)
    out += np.asarray(out_b, np.float32)[None, None, :]
    return out



# revision 28
# speedup vs baseline: 1.3437x; 1.0027x over previous
"""DeepSeekV3 latent attention (MLA) Trainium2 Bass kernel.

Sharding: 8 cores = 2 batches x 4 head-groups (4 heads each).
Each core computes, for its (batch b, head-group hg):
  - c_kv = RMSNorm(x_b @ W_DKV.T) * w        (replicated across the 4 hg cores)
  - k_rope / q projections for its 4 heads (weights sliced on the head axis)
  - causal latent attention (no-max-sub softmax, exp/sum form)
  - out_partial = ctx_hg @ out_w[:, hg_cols].T   (row-parallel partial)
Host sums the 4 partials per batch and adds the bias.

Device layout notes: everything is kept "feature-on-partition, token-on-free"
(transposed) so attention scores come out as S^T [tk, tq] and the probs feed
the PV matmul directly with no on-chip transposes of the big tensors.  The
softmax denominator is computed with a ones-column matmul; RoPE's rotate-half
is a signed 64x64 permutation matmul plus two multiplies.
"""

import numpy as np
import ml_dtypes

import concourse.bass as bass
import concourse.tile as tile
from concourse import bacc
from concourse import mybir
from concourse.bass import ts
from concourse.bass_utils import run_bass_kernel_spmd

BF16 = mybir.dt.bfloat16
F32 = mybir.dt.float32
NPBF16 = ml_dtypes.bfloat16

H, HD, RD, LAT = 16, 128, 64, 512
D_IN = 2048
D_OUT = H * HD
HPC = 4  # heads per core
LC = LAT // 128
EPS = 1e-6
THETA = 10000.0
SCALE = 1.0 / float(np.sqrt(np.float32(HD + RD)))
AF = mybir.ActivationFunctionType


def build_mla_nc(T=2048):
    nc = bacc.Bacc("TRN2", target_bir_lowering=False)
    DC = D_IN // 128      # 16 contraction chunks for the x projections
    NT = T // 128         # 128-token tiles
    NJ = T // 512         # 512-token query supertiles
    LC = LAT // 128       # 4 latent chunks

    # ---------------- I/O (all layouts are host-prepared, partition-major) ---
    xT = nc.dram_tensor("xT", [128, DC, T], BF16, kind="ExternalInput")
    wdkvT = nc.dram_tensor("wdkvT", [128, DC, LAT], BF16, kind="ExternalInput")
    wkrT = nc.dram_tensor("wkrT", [128, DC, HPC * RD], BF16, kind="ExternalInput")
    wqcT = nc.dram_tensor("wqcT", [128, DC, HPC * HD], BF16, kind="ExternalInput")
    wqrT = nc.dram_tensor("wqrT", [128, DC, HPC * RD], BF16, kind="ExternalInput")
    wuk = nc.dram_tensor("wuk", [128, HPC, LAT], BF16, kind="ExternalInput")
    wuvT = nc.dram_tensor("wuvT", [128, HPC, LC, HD], BF16, kind="ExternalInput")
    owT = nc.dram_tensor("owT", [128, HPC, D_OUT], BF16, kind="ExternalInput")
    kvw = nc.dram_tensor("kvw", [128, LAT], F32, kind="ExternalInput")
    cosT = nc.dram_tensor("cosT", [128, T], BF16, kind="ExternalInput")
    sinT = nc.dram_tensor("sinT", [128, T], BF16, kind="ExternalInput")
    perm = nc.dram_tensor("perm", [128, 128], BF16, kind="ExternalInput")
    masks = nc.dram_tensor("masks", [128, 4, 512], BF16, kind="ExternalInput")
    ones16 = nc.dram_tensor("ones16", [128, 128], BF16, kind="ExternalInput")
    out_p = nc.dram_tensor("out_p", [T, D_OUT], BF16, kind="ExternalOutput")

    with tile.TileContext(nc) as tc:
        with (
            tc.tile_pool(name="persist", bufs=1) as persist,
            tc.tile_pool(name="ps_a", bufs=2, space="PSUM") as ps_a,
            tc.tile_pool(name="ps_b", bufs=2, space="PSUM") as ps_b,
            tc.tile_pool(name="ps_pv", bufs=1, space="PSUM") as ps_pv,
        ):
            ckv_nat = persist.tile([128, NT, LAT], BF16)   # [t%128, ttile, lat]
            ckvT = persist.tile([128, LC, T], BF16)        # [lat%128, lc, t]
            kTrot = persist.tile([128, 2, T], BF16)        # [pairrow, h//2, t]
            qTrot = persist.tile([128, 2, T], BF16)
            qcT = persist.tile([128, HPC, T], BF16)        # [hd, h, t]
            ctxT = persist.tile([128, HPC, T], BF16)       # [hd, h, t]

            # ============== Phase A: projections + RMSNorm + RoPE ===========
            with (
                tc.tile_pool(name="projw", bufs=1) as projw,
                tc.tile_pool(name="xs", bufs=2) as xs,
                tc.tile_pool(name="wka", bufs=3) as wka,
            ):
                wdkvT_s = projw.tile([128, DC, LAT], BF16)
                wkrT_s = projw.tile([128, DC, HPC * RD], BF16)
                wqcT_s = projw.tile([128, DC, HPC * HD], BF16)
                wqrT_s = projw.tile([128, DC, HPC * RD], BF16)
                for dc in range(DC):
                    nc.sync.dma_start(wdkvT_s[:, dc, :], wdkvT[:, dc, :])
                for dc in range(DC):
                    nc.sync.dma_start(wkrT_s[:, dc, :], wkrT[:, dc, :])
                    nc.sync.dma_start(wqcT_s[:, dc, :], wqcT[:, dc, :])
                    nc.sync.dma_start(wqrT_s[:, dc, :], wqrT[:, dc, :])
                cos_s = projw.tile([128, T], BF16)
                nc.sync.dma_start(cos_s, cosT[:, :])
                sin_s = projw.tile([128, T], BF16)
                nc.sync.dma_start(sin_s, sinT[:, :])
                perm_s = projw.tile([128, 128], BF16)
                nc.sync.dma_start(perm_s, perm[:, :])
                kvw_s = projw.tile([128, LAT], F32)
                nc.sync.dma_start(kvw_s, kvw[:, :])
                eps_s = projw.tile([128, 1], F32)
                nc.vector.memset(eps_s, EPS)

                def rope_pair(raw_src_psum, dst, rc, jt, tag):
                    # raw_src_psum: [128,512] psum with 2 heads' raw rope rows.
                    raw = wka.tile([128, 512], BF16, tag=f"{tag}_raw")
                    nc.scalar.copy(raw, raw_src_psum)
                    psr = ps_b.tile([128, 512], F32, tag="ps_small")
                    nc.tensor.matmul(psr, lhsT=perm_s, rhs=raw, start=True, stop=True)
                    tmp = wka.tile([128, 512], BF16, tag=f"{tag}_cos")
                    nc.vector.tensor_mul(tmp, raw, cos_s[:, ts(jt, 512)])
                    tmp2 = wka.tile([128, 512], BF16, tag=f"{tag}_sin")
                    nc.vector.tensor_mul(tmp2, psr, sin_s[:, ts(jt, 512)])
                    nc.vector.tensor_add(dst[:, rc, ts(jt, 512)], tmp, tmp2)

                for jt in range(NJ):
                    xts = xs.tile([128, DC, 512], BF16)
                    if jt == 0:
                        # fine-grained so the first contraction chunk lands
                        # as early as possible at kernel start
                        for dcx in range(DC):
                            nc.scalar.dma_start(
                                xts[:, dcx, :], xT[:, dcx, ts(jt, 512)]
                            )
                    else:
                        # later tiles go on the sync queue: their loads gate
                        # on buffer reuse, and the phase-B weight loads must
                        # not queue behind that on the scalar ring
                        xq_eng = nc.scalar if jt == 1 else nc.sync
                        for q4 in range(4):
                            xq_eng.dma_start(
                                xts[:, 4 * q4 : 4 * q4 + 4, :],
                                xT[:, 4 * q4 : 4 * q4 + 4, ts(jt, 512)],
                            )

                    # --- c_kv (natural layout) + RMSNorm ---
                    for tt4 in range(4):
                        tt = jt * 4 + tt4
                        ps = ps_a.tile([128, 512], F32, tag="mm")
                        for dc in range(DC):
                            nc.tensor.matmul(
                                ps,
                                lhsT=xts[:, dc, ts(tt4, 128)],
                                rhs=wdkvT_s[:, dc, :],
                                start=(dc == 0),
                                stop=(dc == DC - 1),
                            )
                        sq = wka.tile([128, LAT], BF16, tag="sq")
                        ssum = wka.tile([128, 1], F32, tag="ssum")
                        nc.scalar.activation(sq, ps, AF.Square, accum_out=ssum)
                        rstd = wka.tile([128, 1], F32, tag="rstd")
                        nc.scalar.activation(
                            rstd, ssum, AF.Sqrt, bias=eps_s, scale=1.0 / LAT
                        )
                        nc.vector.reciprocal(rstd, rstd)
                        tmp = wka.tile([128, LAT], BF16, tag="ckvtmp")
                        nc.scalar.activation(tmp, ps, AF.Copy, scale=rstd)
                        nc.vector.tensor_mul(ckv_nat[:, tt, :], tmp, kvw_s)
                        # transposed copy for the QK side
                        for lc in range(LC):
                            nc.sync.dma_start(
                                ckvT[:, lc, ts(tt, 128)],
                                ckv_nat[:, tt, ts(lc, 128)],
                                transpose=True,
                            )

                    # --- k_rope (transposed) + RoPE ---
                    for rc in range(2):
                        ps = ps_a.tile([128, 512], F32, tag="mm")
                        for dc in range(DC):
                            nc.tensor.matmul(
                                ps,
                                lhsT=wkrT_s[:, dc, ts(rc, 128)],
                                rhs=xts[:, dc, :],
                                start=(dc == 0),
                                stop=(dc == DC - 1),
                            )
                        rope_pair(ps, kTrot, rc, jt, "k")

                    # --- q content (transposed) ---
                    for fc in range(HPC):
                        ps = ps_a.tile([128, 512], F32, tag="mm")
                        for dc in range(DC):
                            nc.tensor.matmul(
                                ps,
                                lhsT=wqcT_s[:, dc, ts(fc, 128)],
                                rhs=xts[:, dc, :],
                                start=(dc == 0),
                                stop=(dc == DC - 1),
                            )
                        nc.vector.tensor_copy(qcT[:, fc, ts(jt, 512)], ps)

                    # --- q rope (transposed) + RoPE ---
                    for rc in range(2):
                        ps = ps_a.tile([128, 512], F32, tag="mm")
                        for dc in range(DC):
                            nc.tensor.matmul(
                                ps,
                                lhsT=wqrT_s[:, dc, ts(rc, 128)],
                                rhs=xts[:, dc, :],
                                start=(dc == 0),
                                stop=(dc == DC - 1),
                            )
                        rope_pair(ps, qTrot, rc, jt, "q")

            # ============== Phase B: attention =============================
            with (
                tc.tile_pool(name="attw", bufs=1) as attw,
                tc.tile_pool(name="qa", bufs=1) as qa_pool,
                tc.tile_pool(name="exps", bufs=10) as exps,
                tc.tile_pool(name="wkb", bufs=3) as wkb,
                tc.tile_pool(name="dnd", bufs=2, space="DRAM") as dnd,
            ):
                wuk_s = attw.tile([128, HPC, LAT], BF16)
                nc.scalar.dma_start(wuk_s, wuk[:, :, :])
                wuvT_s = attw.tile([128, HPC, LC, HD], BF16)
                nc.scalar.dma_start(wuvT_s, wuvT[:, :, :, :])
                owT_s = attw.tile([128, HPC, D_OUT], BF16)
                for hc4 in range(HPC):
                    nc.scalar.dma_start(owT_s[:, hc4, :], owT[:, hc4, :])
                masks_s = attw.tile([128, 4, 512], BF16)
                nc.sync.dma_start(masks_s, masks[:, :, :])
                ones16_s = attw.tile([128, 128], BF16)
                nc.sync.dma_start(ones16_s, ones16[:, :])

                for h in range(HPC):
                    # absorbed q: qa = W_UK_h.T-contract with q_content
                    qaT = qa_pool.tile([128, LC, T], BF16, tag="qaT")
                    for lc in range(LC):
                        for jt in range(NJ):
                            ps = ps_b.tile([128, 512], F32, tag="ps_small")
                            nc.tensor.matmul(
                                ps,
                                lhsT=wuk_s[:, h, ts(lc, 128)],
                                rhs=qcT[:, h, ts(jt, 512)],
                                start=True,
                                stop=True,
                            )
                            if (lc * NJ + jt) % 2 == 0:
                                nc.scalar.copy(qaT[:, lc, ts(jt, 512)], ps)
                            else:
                                nc.vector.tensor_copy(qaT[:, lc, ts(jt, 512)], ps)

                    hb = (h % 2) * 64
                    rc = h // 2
                    for j in range(NJ):
                        ntk = 4 * (j + 1)
                        ps_ctx = ps_pv.tile([128, LC, 512], F32, tag="ps_ctx")
                        exsum = wkb.tile([128, 512], BF16, tag="exsum")
                        def qk_block(tk):
                            # diagonal blocks: queries left of the block's
                            # first key are fully masked -- skip those columns
                            r = tk - 4 * j
                            q0 = 128 * r if r > 0 else 0
                            ps_s = ps_a.tile([128, 512], F32, tag="mm")
                            for lc in range(LC):
                                nc.tensor.matmul(
                                    ps_s[:, q0:],
                                    lhsT=ckvT[:, lc, ts(tk, 128)],
                                    rhs=qaT[:, lc, 512 * j + q0 : 512 * (j + 1)],
                                    start=(lc == 0),
                                    stop=False,
                                )
                            nc.tensor.matmul(
                                ps_s[:, q0:],
                                lhsT=kTrot[hb : hb + 64, rc, ts(tk, 128)],
                                rhs=qTrot[hb : hb + 64, rc, 512 * j + q0 : 512 * (j + 1)],
                                start=False,
                                stop=True,
                            )
                            ex = exps.tile([128, 512], BF16, tag="exp")
                            nc.scalar.activation(ex[:, q0:], ps_s[:, q0:], AF.Exp, scale=SCALE)
                            if r >= 0:
                                nc.gpsimd.tensor_mul(
                                    ex[:, q0:], ex[:, q0:], masks_s[:, r, q0:]
                                )
                            # running key-block sum for the softmax
                            # denominator (idle gpsimd; replaces a per-block
                            # ones-column matmul on the tensor engine)
                            if tk == 0:
                                nc.vector.tensor_copy(exsum, ex)
                            else:
                                nc.vector.tensor_add(
                                    exsum[:, q0:], exsum[:, q0:], ex[:, q0:]
                                )
                            return ex, q0

                        def pv_block(tk, ex, q0):
                            for lc in range(LC):
                                nc.tensor.matmul(
                                    ps_ctx[:, lc, q0:],
                                    lhsT=ckv_nat[:, tk, ts(lc, 128)],
                                    rhs=ex[:, q0:],
                                    start=(tk == 0),
                                    stop=(tk == ntk - 1),
                                )

                        # software pipeline: PV of pair p runs under QK of
                        # pair p+1, giving exp/mask a full QK-pair to finish
                        pend = None
                        for tk2 in range(ntk // 2):
                            ex0, q00 = qk_block(2 * tk2)
                            ex1, q01 = qk_block(2 * tk2 + 1)
                            if pend is not None:
                                pv_block(*pend[0])
                                pv_block(*pend[1])
                            pend = ((2 * tk2, ex0, q00), (2 * tk2 + 1, ex1, q01))
                        pv_block(*pend[0])
                        pv_block(*pend[1])
                        # single cross-partition reduction of the accumulated
                        # exp sums gives the softmax denominator
                        ps_dn = ps_b.tile([1, 512], F32, tag="ps_small")
                        nc.tensor.matmul(
                            ps_dn,
                            lhsT=ones16_s[:, 0:1],
                            rhs=exsum,
                            start=True,
                            stop=True,
                        )
                        # reciprocal on the 1-partition row (fast-approx DVE
                        # op, ~18 bits; denominators are positive sums of
                        # exps), then broadcast via a DRAM round-trip
                        dn_r = wkb.tile([1, 512], F32, tag="dr")
                        nc.vector.reciprocal_approx_fast(out=dn_r, in_=ps_dn)
                        dr_d = dnd.tile([1, 512], F32, tag="dr_d")
                        nc.sync.dma_start(dr_d, dn_r)
                        db = wkb.tile([128, 512], F32, tag="db")
                        nc.gpsimd.dma_start(db, dr_d.to_broadcast((128, 512)))
                        # UV: ctxT_h = W_UV_h.T-contract with ctx_lat
                        cl = wkb.tile([128, LC, 512], BF16, tag="ctxlat")
                        for lc in range(LC):
                            nc.vector.tensor_copy(cl[:, lc, :], ps_ctx[:, lc, :])
                        ps_uv = ps_b.tile([128, 512], F32, tag="ps_small")
                        for lc in range(LC):
                            nc.tensor.matmul(
                                ps_uv,
                                lhsT=wuvT_s[:, h, lc, :],
                                rhs=cl[:, lc, :],
                                start=(lc == 0),
                                stop=(lc == LC - 1),
                            )
                        # drain PSUM fast with a plain copy; the denominator
                        # multiply happens off the critical path once db lands
                        uvr = wkb.tile([128, 512], BF16, tag="uvr")
                        nc.scalar.copy(uvr, ps_uv)
                        nc.gpsimd.tensor_mul(ctxT[:, h, ts(j, 512)], uvr, db)

                # ============== Phase C: output projection =================
                with tc.tile_pool(name="outs", bufs=3) as outs:
                    for tt in range(NT):
                        ot = outs.tile([128, D_OUT], BF16, tag="ot")
                        for oc in range(D_OUT // 512):
                            ps = ps_a.tile([128, 512], F32, tag="mm")
                            for hc in range(HPC):
                                nc.tensor.matmul(
                                    ps,
                                    lhsT=ctxT[:, hc, ts(tt, 128)],
                                    rhs=owT_s[:, hc, ts(oc, 512)],
                                    start=(hc == 0),
                                    stop=(hc == HPC - 1),
                                )
                            if oc % 2 == 0:
                                nc.vector.tensor_copy(ot[:, ts(oc, 512)], ps)
                            else:
                                nc.scalar.copy(ot[:, ts(oc, 512)], ps)
                        nc.sync.dma_start(out_p[ts(tt, 128), :], ot)

    nc.finalize()
    return nc


def _part_major(a2d):
    """[R, C] -> [128, R//128, C] with partition = R % 128."""
    r, c = a2d.shape
    return np.ascontiguousarray(
        a2d.reshape(r // 128, 128, c).transpose(1, 0, 2)
    )


def make_in_maps(x, W_DKV, kv_norm_w, W_KR, W_Q, W_UK, W_UV, out_w, offset, T):
    """Host-side sharding/layout prep. Returns the 8 per-core input dicts."""
    f32 = np.float32
    x = np.asarray(x, f32)
    W_DKV = np.asarray(W_DKV, f32)
    kv_norm_w = np.asarray(kv_norm_w, f32)
    W_KR = np.asarray(W_KR, f32)
    W_Q = np.asarray(W_Q, f32)
    W_UK = np.asarray(W_UK, f32)
    W_UV = np.asarray(W_UV, f32)
    out_w = np.asarray(out_w, f32)
    offset = int(np.asarray(offset))

    def bf(a):
        return np.ascontiguousarray(a).astype(NPBF16)

    # rope tables, mirroring the reference's f32 arithmetic
    inv_freq = (1.0 / (THETA ** (np.arange(0, RD, 2, dtype=f32) / f32(RD)))).astype(f32)
    pos = np.arange(offset, offset + T, dtype=f32)
    ang = (pos[:, None] * inv_freq[None, :]).astype(f32)     # [T, RD/2]
    ang = np.concatenate([ang, ang], axis=-1)                # [T, RD]
    cos_t = np.cos(ang).T                                    # [RD, T]
    sin_t = np.sin(ang).T
    cosT = np.concatenate([cos_t, cos_t], 0)                 # [128, T]
    sinT = np.concatenate([sin_t, sin_t], 0)

    # signed rotate-half permutation (2 heads per 128 partitions), as lhsT
    M = np.zeros((RD, RD), f32)
    for i in range(RD // 2):
        M[i, i + RD // 2] = -1.0
        M[i + RD // 2, i] = 1.0
    perm128 = np.zeros((128, 128), f32)
    perm128[:64, :64] = M
    perm128[64:, 64:] = M
    perm_lhsT = perm128.T

    # diagonal causal masks: block r masked where (128 r + p) > f
    p_idx = np.arange(128)[:, None]
    f_idx = np.arange(512)[None, :]
    masks = np.stack(
        [(128 * r + p_idx <= f_idx).astype(f32) for r in range(4)], axis=1
    )  # [128, 4, 512]

    kvw = np.broadcast_to(kv_norm_w[None, :], (128, LAT)).astype(f32)
    ones16 = np.ones((128, 128), f32)

    wuk_full = W_UK.reshape(H, HD, LAT)
    wuv_full = W_UV.reshape(H, HD, LAT)

    in_maps = []
    for b in range(2):
        xTb = bf(_part_major(x[b].T))  # [128, DC, T]
        for hg in range(4):
            hs = slice(HPC * hg * HD, HPC * (hg + 1) * HD)          # content rows
            rs = slice(D_OUT + HPC * hg * RD, D_OUT + HPC * (hg + 1) * RD)
            heads = slice(HPC * hg, HPC * (hg + 1))
            wuk_c = wuk_full[heads]                                  # [4,128,512]
            wuv_c = wuv_full[heads]
            in_maps.append(
                {
                    "xT": xTb,
                    "wdkvT": bf(_part_major(W_DKV.T)),
                    "wkrT": bf(_part_major(W_KR[HPC * hg * RD : HPC * (hg + 1) * RD].T)),
                    "wqcT": bf(_part_major(W_Q[hs].T)),
                    "wqrT": bf(_part_major(W_Q[rs].T)),
                    "wuk": bf(wuk_c.transpose(1, 0, 2)),             # [128,4,512]
                    "wuvT": bf(
                        wuv_c.transpose(0, 2, 1)                     # [4,512,128]
                        .reshape(HPC, LC, 128, HD)
                        .transpose(2, 0, 1, 3)                       # [128,4,4,128]
                    ),
                    "owT": bf(
                        out_w[:, hs].T.reshape(HPC, 128, D_OUT).transpose(1, 0, 2)
                    ),
                    "kvw": np.ascontiguousarray(kvw),
                    "cosT": bf(cosT),
                    "sinT": bf(sinT),
                    "perm": bf(perm_lhsT),
                    "masks": bf(masks),
                    "ones16": bf(ones16),
                }
            )
    return in_maps


_NC_CACHE = {}


def get_nc(T=2048):
    if T not in _NC_CACHE:
        _NC_CACHE[T] = build_mla_nc(T)
    return _NC_CACHE[T]


LAST_RESULTS = None


def kernel(x, W_DKV, kv_norm_w, W_KR, W_Q, W_UK, W_UV, out_w, out_b, offset):
    global LAST_RESULTS
    import os

    x = np.asarray(x, np.float32)
    B, T, _ = x.shape
    nc = get_nc(T)
    in_maps = make_in_maps(
        x, W_DKV, kv_norm_w, W_KR, W_Q, W_UK, W_UV, out_w, offset, T
    )
    trace = os.environ.get("MLA_TRACE", "0") == "1"
    res = run_bass_kernel_spmd(
        nc, in_maps, core_ids=list(range(8)), trace=trace
    )
    LAST_RESULTS = res
    out = np.zeros((B, T, D_OUT), np.float32)
    for c, r in enumerate(res.results):
        out[c // 4] += np.asarray(r["out_p"], np.float32)
    out += np.asarray(out_b, np.float32)[None, None, :]
    return out



# revision 29
# speedup vs baseline: 1.3611x; 1.0129x over previous
"""DeepSeekV3 latent attention (MLA) Trainium2 Bass kernel.

Sharding: 8 cores = 2 batches x 4 head-groups (4 heads each).
Each core computes, for its (batch b, head-group hg):
  - c_kv = RMSNorm(x_b @ W_DKV.T) * w        (replicated across the 4 hg cores)
  - k_rope / q projections for its 4 heads (weights sliced on the head axis)
  - causal latent attention (no-max-sub softmax, exp/sum form)
  - out_partial = ctx_hg @ out_w[:, hg_cols].T   (row-parallel partial)
Host sums the 4 partials per batch and adds the bias.

Device layout notes: everything is kept "feature-on-partition, token-on-free"
(transposed) so attention scores come out as S^T [tk, tq] and the probs feed
the PV matmul directly with no on-chip transposes of the big tensors.  The
softmax denominator is computed with a ones-column matmul; RoPE's rotate-half
is a signed 64x64 permutation matmul plus two multiplies.
"""

import numpy as np
import ml_dtypes

import concourse.bass as bass
import concourse.tile as tile
from concourse import bacc
from concourse import mybir
from concourse.bass import ts
from concourse.bass_utils import run_bass_kernel_spmd

BF16 = mybir.dt.bfloat16
F32 = mybir.dt.float32
NPBF16 = ml_dtypes.bfloat16

H, HD, RD, LAT = 16, 128, 64, 512
D_IN = 2048
D_OUT = H * HD
HPC = 4  # heads per core
LC = LAT // 128
EPS = 1e-6
THETA = 10000.0
SCALE = 1.0 / float(np.sqrt(np.float32(HD + RD)))
AF = mybir.ActivationFunctionType


def build_mla_nc(T=2048):
    nc = bacc.Bacc("TRN2", target_bir_lowering=False)
    DC = D_IN // 128      # 16 contraction chunks for the x projections
    NT = T // 128         # 128-token tiles
    NJ = T // 512         # 512-token query supertiles
    LC = LAT // 128       # 4 latent chunks

    # ---------------- I/O (all layouts are host-prepared, partition-major) ---
    xT = nc.dram_tensor("xT", [128, DC, T], BF16, kind="ExternalInput")
    wdkvT = nc.dram_tensor("wdkvT", [128, DC, LAT], BF16, kind="ExternalInput")
    wkrT = nc.dram_tensor("wkrT", [128, DC, HPC * RD], BF16, kind="ExternalInput")
    wqcT = nc.dram_tensor("wqcT", [128, DC, HPC * HD], BF16, kind="ExternalInput")
    wqrT = nc.dram_tensor("wqrT", [128, DC, HPC * RD], BF16, kind="ExternalInput")
    wuk = nc.dram_tensor("wuk", [128, HPC, LAT], BF16, kind="ExternalInput")
    wuvT = nc.dram_tensor("wuvT", [128, HPC, LC, HD], BF16, kind="ExternalInput")
    owT = nc.dram_tensor("owT", [128, HPC, D_OUT], BF16, kind="ExternalInput")
    kvw = nc.dram_tensor("kvw", [128, LAT], F32, kind="ExternalInput")
    cosT = nc.dram_tensor("cosT", [128, T], BF16, kind="ExternalInput")
    sinT = nc.dram_tensor("sinT", [128, T], BF16, kind="ExternalInput")
    perm = nc.dram_tensor("perm", [128, 128], BF16, kind="ExternalInput")
    masks = nc.dram_tensor("masks", [128, 4, 512], BF16, kind="ExternalInput")
    ones16 = nc.dram_tensor("ones16", [128, 128], BF16, kind="ExternalInput")
    out_p = nc.dram_tensor("out_p", [T, D_OUT], BF16, kind="ExternalOutput")

    with tile.TileContext(nc) as tc:
        with (
            tc.tile_pool(name="persist", bufs=1) as persist,
            tc.tile_pool(name="ps_a", bufs=2, space="PSUM") as ps_a,
            tc.tile_pool(name="ps_b", bufs=2, space="PSUM") as ps_b,
            tc.tile_pool(name="ps_pv", bufs=1, space="PSUM") as ps_pv,
        ):
            ckv_nat = persist.tile([128, NT, LAT], BF16)   # [t%128, ttile, lat]
            ckvT = persist.tile([128, LC, T], BF16)        # [lat%128, lc, t]
            kTrot = persist.tile([128, 2, T], BF16)        # [pairrow, h//2, t]
            qTrot = persist.tile([128, 2, T], BF16)
            qcT = persist.tile([128, HPC, T], BF16)        # [hd, h, t]
            ctxT = persist.tile([128, HPC, T], BF16)       # [hd, h, t]

            # ============== Phase A: projections + RMSNorm + RoPE ===========
            with (
                tc.tile_pool(name="projw", bufs=1) as projw,
                tc.tile_pool(name="xs", bufs=2) as xs,
                tc.tile_pool(name="wka", bufs=3) as wka,
            ):
                wdkvT_s = projw.tile([128, DC, LAT], BF16)
                wkrT_s = projw.tile([128, DC, HPC * RD], BF16)
                wqcT_s = projw.tile([128, DC, HPC * HD], BF16)
                wqrT_s = projw.tile([128, DC, HPC * RD], BF16)
                for dc in range(DC):
                    nc.sync.dma_start(wdkvT_s[:, dc, :], wdkvT[:, dc, :])
                for dc in range(DC):
                    nc.sync.dma_start(wkrT_s[:, dc, :], wkrT[:, dc, :])
                    nc.sync.dma_start(wqcT_s[:, dc, :], wqcT[:, dc, :])
                    nc.sync.dma_start(wqrT_s[:, dc, :], wqrT[:, dc, :])
                cos_s = projw.tile([128, T], BF16)
                nc.sync.dma_start(cos_s, cosT[:, :])
                sin_s = projw.tile([128, T], BF16)
                nc.sync.dma_start(sin_s, sinT[:, :])
                perm_s = projw.tile([128, 128], BF16)
                nc.sync.dma_start(perm_s, perm[:, :])
                kvw_s = projw.tile([128, LAT], F32)
                nc.sync.dma_start(kvw_s, kvw[:, :])
                eps_s = projw.tile([128, 1], F32)
                nc.vector.memset(eps_s, EPS)

                def rope_pair(raw_src_psum, dst, rc, jt, tag):
                    # raw_src_psum: [128,512] psum with 2 heads' raw rope rows.
                    raw = wka.tile([128, 512], BF16, tag=f"{tag}_raw")
                    nc.scalar.copy(raw, raw_src_psum)
                    psr = ps_b.tile([128, 512], F32, tag="ps_small")
                    nc.tensor.matmul(psr, lhsT=perm_s, rhs=raw, start=True, stop=True)
                    tmp = wka.tile([128, 512], BF16, tag=f"{tag}_cos")
                    nc.vector.tensor_mul(tmp, raw, cos_s[:, ts(jt, 512)])
                    tmp2 = wka.tile([128, 512], BF16, tag=f"{tag}_sin")
                    nc.vector.tensor_mul(tmp2, psr, sin_s[:, ts(jt, 512)])
                    nc.vector.tensor_add(dst[:, rc, ts(jt, 512)], tmp, tmp2)

                for jt in range(NJ):
                    xts = xs.tile([128, DC, 512], BF16)
                    if jt == 0:
                        # fine-grained so the first contraction chunk lands
                        # as early as possible at kernel start
                        for dcx in range(DC):
                            nc.scalar.dma_start(
                                xts[:, dcx, :], xT[:, dcx, ts(jt, 512)]
                            )
                    else:
                        # later tiles go on the sync queue: their loads gate
                        # on buffer reuse, and the phase-B weight loads must
                        # not queue behind that on the scalar ring
                        xq_eng = nc.scalar if jt == 1 else nc.sync
                        for q4 in range(4):
                            xq_eng.dma_start(
                                xts[:, 4 * q4 : 4 * q4 + 4, :],
                                xT[:, 4 * q4 : 4 * q4 + 4, ts(jt, 512)],
                            )

                    # --- c_kv (natural layout) + RMSNorm ---
                    for tt4 in range(4):
                        tt = jt * 4 + tt4
                        ps = ps_a.tile([128, 512], F32, tag="mm")
                        for dc in range(DC):
                            nc.tensor.matmul(
                                ps,
                                lhsT=xts[:, dc, ts(tt4, 128)],
                                rhs=wdkvT_s[:, dc, :],
                                start=(dc == 0),
                                stop=(dc == DC - 1),
                            )
                        sq = wka.tile([128, LAT], BF16, tag="sq")
                        ssum = wka.tile([128, 1], F32, tag="ssum")
                        nc.scalar.activation(sq, ps, AF.Square, accum_out=ssum)
                        rstd = wka.tile([128, 1], F32, tag="rstd")
                        nc.scalar.activation(
                            rstd, ssum, AF.Sqrt, bias=eps_s, scale=1.0 / LAT
                        )
                        nc.vector.reciprocal(rstd, rstd)
                        tmp = wka.tile([128, LAT], BF16, tag="ckvtmp")
                        nc.scalar.activation(tmp, ps, AF.Copy, scale=rstd)
                        nc.vector.tensor_mul(ckv_nat[:, tt, :], tmp, kvw_s)
                        # transposed copy for the QK side
                        for lc in range(LC):
                            nc.sync.dma_start(
                                ckvT[:, lc, ts(tt, 128)],
                                ckv_nat[:, tt, ts(lc, 128)],
                                transpose=True,
                            )

                    # --- k_rope (transposed) + RoPE ---
                    for rc in range(2):
                        ps = ps_a.tile([128, 512], F32, tag="mm")
                        for dc in range(DC):
                            nc.tensor.matmul(
                                ps,
                                lhsT=wkrT_s[:, dc, ts(rc, 128)],
                                rhs=xts[:, dc, :],
                                start=(dc == 0),
                                stop=(dc == DC - 1),
                            )
                        rope_pair(ps, kTrot, rc, jt, "k")

                    # --- q content (transposed) ---
                    for fc in range(HPC):
                        ps = ps_a.tile([128, 512], F32, tag="mm")
                        for dc in range(DC):
                            nc.tensor.matmul(
                                ps,
                                lhsT=wqcT_s[:, dc, ts(fc, 128)],
                                rhs=xts[:, dc, :],
                                start=(dc == 0),
                                stop=(dc == DC - 1),
                            )
                        nc.vector.tensor_copy(qcT[:, fc, ts(jt, 512)], ps)

                    # --- q rope (transposed) + RoPE ---
                    for rc in range(2):
                        ps = ps_a.tile([128, 512], F32, tag="mm")
                        for dc in range(DC):
                            nc.tensor.matmul(
                                ps,
                                lhsT=wqrT_s[:, dc, ts(rc, 128)],
                                rhs=xts[:, dc, :],
                                start=(dc == 0),
                                stop=(dc == DC - 1),
                            )
                        rope_pair(ps, qTrot, rc, jt, "q")

            # ============== Phase B: attention =============================
            with (
                tc.tile_pool(name="attw", bufs=1) as attw,
                tc.tile_pool(name="qa", bufs=1) as qa_pool,
                tc.tile_pool(name="exps", bufs=10) as exps,
                tc.tile_pool(name="wkb", bufs=3) as wkb,
                tc.tile_pool(name="dnd", bufs=2, space="DRAM") as dnd,
            ):
                wuk_s = attw.tile([128, HPC, LAT], BF16)
                nc.scalar.dma_start(wuk_s, wuk[:, :, :])
                wuvT_s = attw.tile([128, HPC, LC, HD], BF16)
                nc.scalar.dma_start(wuvT_s, wuvT[:, :, :, :])
                owT_s = attw.tile([128, HPC, D_OUT], BF16)
                for hc4 in range(HPC):
                    nc.scalar.dma_start(owT_s[:, hc4, :], owT[:, hc4, :])
                masks_s = attw.tile([128, 4, 512], BF16)
                nc.sync.dma_start(masks_s, masks[:, :, :])
                ones16_s = attw.tile([128, 128], BF16)
                nc.sync.dma_start(ones16_s, ones16[:, :])

                def emit_tail(ps_ctx_t, exsum_t, j_t, h_t):
                    # deferred j-tail: denominator reduce, PSUM drains and
                    # UV projection, interleaved into the next j's QK stream
                    ps_dn = ps_b.tile([1, 512], F32, tag="ps_small")
                    nc.tensor.matmul(
                        ps_dn,
                        lhsT=ones16_s[:, 0:1],
                        rhs=exsum_t,
                        start=True,
                        stop=True,
                    )
                    dn_r = wkb.tile([1, 512], F32, tag="dr")
                    nc.vector.reciprocal_approx_fast(out=dn_r, in_=ps_dn)
                    dr_d = dnd.tile([1, 512], F32, tag="dr_d")
                    nc.sync.dma_start(dr_d, dn_r)
                    db = wkb.tile([128, 512], F32, tag="db")
                    nc.gpsimd.dma_start(db, dr_d.to_broadcast((128, 512)))
                    cl = wkb.tile([128, LC, 512], BF16, tag="ctxlat")
                    for lc in range(LC):
                        nc.vector.tensor_copy(cl[:, lc, :], ps_ctx_t[:, lc, :])
                    ps_uv = ps_b.tile([128, 512], F32, tag="ps_small")
                    for lc in range(LC):
                        nc.tensor.matmul(
                            ps_uv,
                            lhsT=wuvT_s[:, h_t, lc, :],
                            rhs=cl[:, lc, :],
                            start=(lc == 0),
                            stop=(lc == LC - 1),
                        )
                    uvr = wkb.tile([128, 512], BF16, tag="uvr")
                    nc.scalar.copy(uvr, ps_uv)
                    nc.gpsimd.tensor_mul(ctxT[:, h_t, ts(j_t, 512)], uvr, db)

                tail_prev = None
                for h in range(HPC):
                    # absorbed q: qa = W_UK_h.T-contract with q_content
                    qaT = qa_pool.tile([128, LC, T], BF16, tag="qaT")
                    for lc in range(LC):
                        for jt in range(NJ):
                            ps = ps_b.tile([128, 512], F32, tag="ps_small")
                            nc.tensor.matmul(
                                ps,
                                lhsT=wuk_s[:, h, ts(lc, 128)],
                                rhs=qcT[:, h, ts(jt, 512)],
                                start=True,
                                stop=True,
                            )
                            if (lc * NJ + jt) % 2 == 0:
                                nc.scalar.copy(qaT[:, lc, ts(jt, 512)], ps)
                            else:
                                nc.vector.tensor_copy(qaT[:, lc, ts(jt, 512)], ps)

                    hb = (h % 2) * 64
                    rc = h // 2
                    for j in range(NJ):
                        ntk = 4 * (j + 1)
                        ps_ctx = ps_pv.tile([128, LC, 512], F32, tag="ps_ctx")
                        exsum = wkb.tile([128, 512], BF16, tag="exsum")
                        def qk_block(tk):
                            # diagonal blocks: queries left of the block's
                            # first key are fully masked -- skip those columns
                            r = tk - 4 * j
                            q0 = 128 * r if r > 0 else 0
                            ps_s = ps_a.tile([128, 512], F32, tag="mm")
                            for lc in range(LC):
                                nc.tensor.matmul(
                                    ps_s[:, q0:],
                                    lhsT=ckvT[:, lc, ts(tk, 128)],
                                    rhs=qaT[:, lc, 512 * j + q0 : 512 * (j + 1)],
                                    start=(lc == 0),
                                    stop=False,
                                )
                            nc.tensor.matmul(
                                ps_s[:, q0:],
                                lhsT=kTrot[hb : hb + 64, rc, ts(tk, 128)],
                                rhs=qTrot[hb : hb + 64, rc, 512 * j + q0 : 512 * (j + 1)],
                                start=False,
                                stop=True,
                            )
                            ex = exps.tile([128, 512], BF16, tag="exp")
                            nc.scalar.activation(ex[:, q0:], ps_s[:, q0:], AF.Exp, scale=SCALE)
                            if r >= 0:
                                nc.gpsimd.tensor_mul(
                                    ex[:, q0:], ex[:, q0:], masks_s[:, r, q0:]
                                )
                            # running key-block sum for the softmax
                            # denominator (idle gpsimd; replaces a per-block
                            # ones-column matmul on the tensor engine)
                            if tk == 0:
                                nc.vector.tensor_copy(exsum, ex)
                            else:
                                nc.vector.tensor_add(
                                    exsum[:, q0:], exsum[:, q0:], ex[:, q0:]
                                )
                            return ex, q0

                        def pv_block(tk, ex, q0):
                            for lc in range(LC):
                                nc.tensor.matmul(
                                    ps_ctx[:, lc, q0:],
                                    lhsT=ckv_nat[:, tk, ts(lc, 128)],
                                    rhs=ex[:, q0:],
                                    start=(tk == 0),
                                    stop=(tk == ntk - 1),
                                )

                        # software pipeline: PV of pair p runs under QK of
                        # pair p+1, giving exp/mask a full QK-pair to finish
                        pend = None
                        for tk2 in range(ntk // 2):
                            ex0, q00 = qk_block(2 * tk2)
                            ex1, q01 = qk_block(2 * tk2 + 1)
                            if tk2 == 1 and tail_prev is not None:
                                emit_tail(*tail_prev)
                                tail_prev = None
                            if pend is not None:
                                pv_block(*pend[0])
                                pv_block(*pend[1])
                            pend = ((2 * tk2, ex0, q00), (2 * tk2 + 1, ex1, q01))
                        pv_block(*pend[0])
                        pv_block(*pend[1])
                        tail_prev = (ps_ctx, exsum, j, h)

                emit_tail(*tail_prev)
                tail_prev = None

                # ============== Phase C: output projection =================
                with tc.tile_pool(name="outs", bufs=3) as outs:
                    for tt in range(NT):
                        ot = outs.tile([128, D_OUT], BF16, tag="ot")
                        for oc in range(D_OUT // 512):
                            ps = ps_a.tile([128, 512], F32, tag="mm")
                            for hc in range(HPC):
                                nc.tensor.matmul(
                                    ps,
                                    lhsT=ctxT[:, hc, ts(tt, 128)],
                                    rhs=owT_s[:, hc, ts(oc, 512)],
                                    start=(hc == 0),
                                    stop=(hc == HPC - 1),
                                )
                            if oc % 2 == 0:
                                nc.vector.tensor_copy(ot[:, ts(oc, 512)], ps)
                            else:
                                nc.scalar.copy(ot[:, ts(oc, 512)], ps)
                        nc.sync.dma_start(out_p[ts(tt, 128), :], ot)

    nc.finalize()
    return nc


def _part_major(a2d):
    """[R, C] -> [128, R//128, C] with partition = R % 128."""
    r, c = a2d.shape
    return np.ascontiguousarray(
        a2d.reshape(r // 128, 128, c).transpose(1, 0, 2)
    )


def make_in_maps(x, W_DKV, kv_norm_w, W_KR, W_Q, W_UK, W_UV, out_w, offset, T):
    """Host-side sharding/layout prep. Returns the 8 per-core input dicts."""
    f32 = np.float32
    x = np.asarray(x, f32)
    W_DKV = np.asarray(W_DKV, f32)
    kv_norm_w = np.asarray(kv_norm_w, f32)
    W_KR = np.asarray(W_KR, f32)
    W_Q = np.asarray(W_Q, f32)
    W_UK = np.asarray(W_UK, f32)
    W_UV = np.asarray(W_UV, f32)
    out_w = np.asarray(out_w, f32)
    offset = int(np.asarray(offset))

    def bf(a):
        return np.ascontiguousarray(a).astype(NPBF16)

    # rope tables, mirroring the reference's f32 arithmetic
    inv_freq = (1.0 / (THETA ** (np.arange(0, RD, 2, dtype=f32) / f32(RD)))).astype(f32)
    pos = np.arange(offset, offset + T, dtype=f32)
    ang = (pos[:, None] * inv_freq[None, :]).astype(f32)     # [T, RD/2]
    ang = np.concatenate([ang, ang], axis=-1)                # [T, RD]
    cos_t = np.cos(ang).T                                    # [RD, T]
    sin_t = np.sin(ang).T
    cosT = np.concatenate([cos_t, cos_t], 0)                 # [128, T]
    sinT = np.concatenate([sin_t, sin_t], 0)

    # signed rotate-half permutation (2 heads per 128 partitions), as lhsT
    M = np.zeros((RD, RD), f32)
    for i in range(RD // 2):
        M[i, i + RD // 2] = -1.0
        M[i + RD // 2, i] = 1.0
    perm128 = np.zeros((128, 128), f32)
    perm128[:64, :64] = M
    perm128[64:, 64:] = M
    perm_lhsT = perm128.T

    # diagonal causal masks: block r masked where (128 r + p) > f
    p_idx = np.arange(128)[:, None]
    f_idx = np.arange(512)[None, :]
    masks = np.stack(
        [(128 * r + p_idx <= f_idx).astype(f32) for r in range(4)], axis=1
    )  # [128, 4, 512]

    kvw = np.broadcast_to(kv_norm_w[None, :], (128, LAT)).astype(f32)
    ones16 = np.ones((128, 128), f32)

    wuk_full = W_UK.reshape(H, HD, LAT)
    wuv_full = W_UV.reshape(H, HD, LAT)

    in_maps = []
    for b in range(2):
        xTb = bf(_part_major(x[b].T))  # [128, DC, T]
        for hg in range(4):
            hs = slice(HPC * hg * HD, HPC * (hg + 1) * HD)          # content rows
            rs = slice(D_OUT + HPC * hg * RD, D_OUT + HPC * (hg + 1) * RD)
            heads = slice(HPC * hg, HPC * (hg + 1))
            wuk_c = wuk_full[heads]                                  # [4,128,512]
            wuv_c = wuv_full[heads]
            in_maps.append(
                {
                    "xT": xTb,
                    "wdkvT": bf(_part_major(W_DKV.T)),
                    "wkrT": bf(_part_major(W_KR[HPC * hg * RD : HPC * (hg + 1) * RD].T)),
                    "wqcT": bf(_part_major(W_Q[hs].T)),
                    "wqrT": bf(_part_major(W_Q[rs].T)),
                    "wuk": bf(wuk_c.transpose(1, 0, 2)),             # [128,4,512]
                    "wuvT": bf(
                        wuv_c.transpose(0, 2, 1)                     # [4,512,128]
                        .reshape(HPC, LC, 128, HD)
                        .transpose(2, 0, 1, 3)                       # [128,4,4,128]
                    ),
                    "owT": bf(
                        out_w[:, hs].T.reshape(HPC, 128, D_OUT).transpose(1, 0, 2)
                    ),
                    "kvw": np.ascontiguousarray(kvw),
                    "cosT": bf(cosT),
                    "sinT": bf(sinT),
                    "perm": bf(perm_lhsT),
                    "masks": bf(masks),
                    "ones16": bf(ones16),
                }
            )
    return in_maps


_NC_CACHE = {}


def get_nc(T=2048):
    if T not in _NC_CACHE:
        _NC_CACHE[T] = build_mla_nc(T)
    return _NC_CACHE[T]


LAST_RESULTS = None


def kernel(x, W_DKV, kv_norm_w, W_KR, W_Q, W_UK, W_UV, out_w, out_b, offset):
    global LAST_RESULTS
    import os

    x = np.asarray(x, np.float32)
    B, T, _ = x.shape
    nc = get_nc(T)
    in_maps = make_in_maps(
        x, W_DKV, kv_norm_w, W_KR, W_Q, W_UK, W_UV, out_w, offset, T
    )
    trace = os.environ.get("MLA_TRACE", "0") == "1"
    res = run_bass_kernel_spmd(
        nc, in_maps, core_ids=list(range(8)), trace=trace
    )
    LAST_RESULTS = res
    out = np.zeros((B, T, D_OUT), np.float32)
    for c, r in enumerate(res.results):
        out[c // 4] += np.asarray(r["out_p"], np.float32)
    out += np.asarray(out_b, np.float32)[None, None, :]
    return out

